# revision 20
# baseline (speedup 1.0000x reference)
"""GATv2 x2 + residual on 8 TRN2 NeuronCores (Bass/Tile).

Strategy (self-contained; N=100000, D=64, E=1700000):

- Nodes are assigned to the 8 cores by a greedy balancer (12500 each),
  then ordered per-core by descending in-degree ("device order"). All
  device tables (strips, nd accumulators, output) use this order, so
  per-destination loads are plain sequential DMAs instead of gathers.
- |att| is folded into the weights and features are permuted so att>0
  features come first. The per-edge score uses the exact identity
  lrelu_0.2(z) = 0.6 z + 0.4|z|:
      score = 0.6 (a_src + b_dst) + 0.4 (sum_pos|z| - sum_neg|z|)
  where a = sum_pos xl - sum_neg xl and b likewise for xr are per-node
  scalars computed once in the matmul phase and carried as column 64 of
  the fp16 strips. This removes the per-slot leaky-relu pass entirely;
  the |.| sums use tensor_reduce(apply_absolute_value=True).
- All per-edge tensors are fp16 (2x DVE throughput on packed ops); the
  weighted message sum uses the z-trick
      num = sum ex*z - den*xr            (z = xl + xr)
  so the gathered xl tile is only read once (by the z add).
- Tables are fp16 with 128-wide rows ([xl(64) | a | pad]): one 256B
  gather descriptor per edge, same as f32/64-wide. Gathers cycle over 4
  SWDGE queues with a 64KB descriptor ring.
- Self-loops never enter the edge streams; slot padding gathers a
  poison row (a = -1e4) whose score underflows exp to exactly 0.
- Per-row softmax over destination-major slots [128 dst x slots];
  heavy rows overflow to per-core compacted virtual rows (nd_v).
  Virtual groups run FIRST; each main group is immediately followed by
  its merge, and (in layer 0) by the layer-1 matmul chunk for the same
  columns, removing the serial merge/matmul phases.
- Layer 1 merge writes PE-transposed features into hT for layer 2's
  matmuls; layer 2 merge writes the final features. Host adds residual.
"""
import os
import numpy as np

N = 100000
D = 64
M = 8
NS = 12500              # real nodes per core
PC = 12544              # padded strip rows (98 * 128)
NW = 4                  # gather windows (core pairs)
WIN = 2 * PC            # rows per window
NEG = 0.2
CAPQ = 0.75             # slot cap quantile within a group
G = 4                   # 128-row blocks per group
RG = 128 * G            # rows per group (512)
NGM = 25                # main groups (25*512 = 12800 >= NS)
MROWS = NGM * RG        # 12800
TBL = M * PC            # table rows (100352)
POISON_A = -10000.0
PIDX = NS               # window-local poison row (even strip, row NS)


# ----------------------------------------------------------------------
# host preprocessing
# ----------------------------------------------------------------------

def _assign_cores(src, dst):
    order = np.argsort(src, kind='stable')
    d_sorted = dst[order]
    starts = np.searchsorted(src[order], np.arange(N + 1))
    core = np.full(N, -1, np.int32)
    quota = np.full(M, NS, np.int64)
    cnt = np.zeros((N, NW), np.float32)
    outdeg = starts[1:] - starts[:-1]
    proc = np.argsort(-outdeg, kind='stable')
    pair_edges = np.zeros(NW, np.float64)
    for v in proc:
        ds = d_sorted[starts[v]:starts[v + 1]]
        costs = cnt[ds].sum(axis=0) if len(ds) else np.zeros(NW, np.float32)
        costs = costs + 1e-7 * pair_edges
        best = None
        for p in np.argsort(costs, kind='stable'):
            if quota[2 * p] > 0 or quota[2 * p + 1] > 0:
                best = int(p)
                break
        c0, c1 = 2 * best, 2 * best + 1
        c = c0 if quota[c0] >= quota[c1] else c1
        core[v] = c
        quota[c] -= 1
        if len(ds):
            cnt[ds, best] += 1.0
            pair_edges[best] += len(ds)
    return core


def _wrap_idx(flat):
    """dma_gather index layout: [128, n/16] int16, 16-wrapped, 8x replicated."""
    n = len(flat)
    assert n % 16 == 0
    w = flat.reshape(n // 16, 16).T
    return np.ascontiguousarray(np.tile(w, (8, 1)), dtype=np.int16)


def preprocess(x, edge_index):
    src, dst = np.asarray(edge_index[0]), np.asarray(edge_index[1])
    nonself = src != dst
    src, dst = src[nonself].astype(np.int64), dst[nonself].astype(np.int64)
    core = _assign_cores(np.asarray(edge_index[0]), np.asarray(edge_index[1]))
    win_of = core // 2

    # provisional per-core rank (any order), then degree-sort -> device order
    node_order = np.zeros((M, NS), np.int64)
    for m in range(M):
        ids = np.where(core == m)[0]
        node_order[m] = ids
    # per-node non-self in-degree
    indeg = np.bincount(dst, minlength=N)
    for m in range(M):
        o = np.argsort(-indeg[node_order[m]], kind='stable')
        node_order[m] = node_order[m][o]
    rank = np.zeros(N, np.int64)
    for m in range(M):
        rank[node_order[m]] = np.arange(NS)
    twl = (core % 2).astype(np.int64) * PC + rank       # window-local table row

    # per-core edge lists grouped by (dst rank, window)
    svals, bounds = [], []
    for m in range(M):
        em = np.where(core[dst] == m)[0]
        es, ed = src[em], dst[em]
        key = rank[ed] * NW + win_of[es]
        # ascending src within each (rank, window) segment: gather columns
        # then cluster into narrow table ranges (better HBM locality)
        ko = np.lexsort((twl[es], key))
        svals.append(twl[es[ko]])
        bounds.append(np.searchsorted(key[ko], np.arange(NS * NW + 1)))

    cnts = np.zeros((M, NS, NW), np.int64)
    for m in range(M):
        b = bounds[m]
        cnts[m] = (b[1:] - b[:-1]).reshape(NS, NW)

    # shared main-group slot caps (device order = degree order)
    S_main = np.zeros((NGM, NW), np.int32)
    for g in range(NGM):
        caps = np.zeros((M, NW), np.int32)
        for m in range(M):
            rows = cnts[m][g * RG:(g + 1) * RG]
            if len(rows) == 0:
                continue
            caps[m] = np.ceil(np.quantile(rows, CAPQ, axis=0)).astype(np.int32)
        S_main[g] = caps.max(axis=0)
    S_main = np.maximum(S_main, 1)

    # overflow -> per-core compacted virtual rows
    gidx = np.minimum(np.arange(NS) // RG, NGM - 1)
    ov = np.maximum(cnts - S_main[gidx][None, :, :], 0)   # [M, NS, NW]
    virt = []
    for m in range(M):
        v = np.where(ov[m].sum(axis=1) > 0)[0]
        virt.append(v[np.argsort(-ov[m][v].sum(axis=1), kind='stable')])
    NV = max(len(v) for v in virt)
    NGV = (NV + RG - 1) // RG
    S_virt = np.zeros((NGV, NW), np.int32)
    for g in range(NGV):
        mx = np.zeros(NW, np.int32)
        for m in range(M):
            v = virt[m][g * RG:(g + 1) * RG]
            if len(v):
                mx = np.maximum(mx, ov[m][v].max(axis=0))
        S_virt[g] = np.maximum(mx, 1)

    NG = NGM + NGV
    S = np.concatenate([S_main, S_virt], axis=0)         # [NG, NW]
    R = NG * RG
    ZR = R                                               # zero-row id

    hv_glob = np.zeros(NGM, bool)
    percore = []
    for m in range(M):
        b = bounds[m]
        sv = svals[m]
        vr = virt[m]
        virtrow = np.full(NS, NGV * RG, np.int64)  # default: zero row (virt-local)
        sidx = [[] for _ in range(NW)]
        vxidx = []
        for g in range(NG):
            is_v = g >= NGM
            for c in range(NW):
                S_c = int(S[g, c])
                flat = np.full(128 * G * S_c, PIDX, np.int16)
                for gg in range(G):
                    for p in range(128):
                        i = gg * 128 + p
                        if not is_v:
                            r = g * RG + i
                            if r >= NS:
                                continue
                        else:
                            gi = (g - NGM) * RG + i
                            if gi >= len(vr):
                                continue
                            r = vr[gi]
                            if c == 0:
                                virtrow[r] = (g - NGM) * RG + i
                        lo, hi = b[r * NW + c], b[r * NW + c + 1]
                        if is_v:
                            lo = lo + int(S_main[min(r // RG, NGM - 1), c])
                        seg = sv[lo:min(hi, lo + S_c)]
                        for s_i, v_ in enumerate(seg):
                            flat[(gg * S_c + s_i) * 128 + p] = v_
                sidx[c].append(_wrap_idx(flat))
            if is_v:
                vx = np.zeros(RG, np.int16)
                gi0 = (g - NGM) * RG
                for i in range(RG):
                    gi = gi0 + i
                    vx[i] = vr[gi] if gi < len(vr) else NS  # pad: xr row NS
                vxidx.append(_wrap_idx(vx))
        mB = np.full(MROWS, NGV * RG, np.int16)
        mB[:NS] = virtrow
        vranks = np.where(virtrow != NGV * RG)[0]
        hv_glob[np.minimum(vranks // RG, NGM - 1)] = True
        percore.append(dict(
            sidx=[np.concatenate(s, axis=1) for s in sidx],
            vxidx=(np.concatenate(vxidx, axis=1) if NGV else
                   np.zeros((128, 32), np.int16)),
            mB=np.concatenate([_wrap_idx(mB[g * RG:(g + 1) * RG])
                               for g in range(NGM)], axis=1),
        ))
    return dict(core=core, node_order=node_order, S=S, NGV=NGV, NG=NG,
                R=R, percore=percore, hv=hv_glob.tolist())


# ----------------------------------------------------------------------
# device program
# ----------------------------------------------------------------------

def build_program(S, NGV, NPOS, HV=None):
    from concourse import bass, mybir, tile
    from concourse import bacc
    f32 = mybir.dt.float32
    f16 = mybir.dt.float16
    i16 = mybir.dt.int16
    NG = NGM + NGV
    R = NG * RG
    CC = [sum(8 * G * int(S[g][c]) for g in range(NG)) for c in range(NW)]
    assert 0 < NPOS < 64
    if HV is None:
        HV = [True] * NGM

    nc = bacc.Bacc(num_swdge_queues=4, dynamic_dma_scratch_size=64512)
    P = {}
    P['xT0'] = nc.declare_dram_parameter("xT0", [65, PC], f16, isOutput=False)
    P['Wcat0'] = nc.declare_dram_parameter("Wcat0", [65, 128], f16, isOutput=False)
    P['Wcat1'] = nc.declare_dram_parameter("Wcat1", [65, 128], f32, isOutput=False)
    P['bias0'] = nc.declare_dram_parameter("bias0", [128, 64], f32, isOutput=False)
    P['bias1'] = nc.declare_dram_parameter("bias1", [128, 64], f32, isOutput=False)
    P['rat'] = nc.declare_dram_parameter("rat", [128, 64], f32, isOutput=False)
    P['ident'] = nc.declare_dram_parameter("ident", [128, 128], f32, isOutput=False)
    P['poison'] = nc.declare_dram_parameter("poison", [1, 128], f16, isOutput=False)
    for c in range(NW):
        P[f'sidx{c}'] = nc.declare_dram_parameter(f"sidx{c}", [128, CC[c]], i16, isOutput=False)
    P['vxidx'] = nc.declare_dram_parameter("vxidx", [128, max(32 * NGV, 32)], i16, isOutput=False)
    P['mB'] = nc.declare_dram_parameter("mB", [128, 32 * NGM], i16, isOutput=False)
    h2out = nc.declare_dram_parameter("h2", [MROWS, 64], f32, isOutput=True)

    strip = [nc.dram_tensor(f"strip{l}", [MROWS, 128], f16) for l in range(2)]
    xr_t = [nc.dram_tensor(f"xr{l}", [MROWS, 128], f16) for l in range(2)]
    table = [nc.dram_tensor(f"table{l}", [TBL, 128], f16, addr_space="Shared")
             for l in range(2)]
    nd_v = [nc.dram_tensor(f"ndv{l}", [NGV * RG + 128, 128], f16) for l in range(2)]

    from contextlib import ExitStack
    _regs = {}

    def nireg(v):
        if v not in _regs:
            r = nc.gpsimd.alloc_register(f"ni{v}")
            nc.gpsimd.reg_mov(r, v)
            _regs[v] = r
        return _regs[v]

    _qctr = [0]

    def nextq():
        q = _qctr[0] % 4
        _qctr[0] += 1
        return q

    with tile.TileContext(nc) as tc, ExitStack() as es, \
            nc.allow_low_precision(reason="fp16 softmax accumulators"):
        cpool = es.enter_context(tc.tile_pool(name="const", bufs=1))
        wcat = [cpool.tile([65, 128], f16 if i == 0 else f32, name=f"wcat{i}")
                for i in range(2)]
        biasT = [cpool.tile([128, 64], f32, name=f"biasT{i}") for i in range(2)]
        rat = cpool.tile([128, 64], f32)
        ident = cpool.tile([128, 128], f32)
        poison = cpool.tile([1, 128], f16)
        for l in range(2):
            nc.sync.dma_start(out=wcat[l][:], in_=P[f'Wcat{l}'][:, :])
            nc.sync.dma_start(out=biasT[l][:], in_=P[f'bias{l}'][:, :])
        nc.sync.dma_start(out=rat[:], in_=P['rat'][:, :])
        nc.sync.dma_start(out=ident[:], in_=P['ident'][:, :])
        nc.sync.dma_start(out=poison[:], in_=P['poison'][:, :])
        # zero-rows of nd tables (fp16); ones-row of hT
        zt = cpool.tile([128, 128], f16)
        nc.vector.memset(zt[:], 0.0)
        zt2 = cpool.tile([128, 256], f16)
        nc.vector.memset(zt2[:], 0.0)
        for l in range(2):
            nc.sync.dma_start(out=nd_v[l][NGV * RG:NGV * RG + 128, :], in_=zt[:])
            # zero tail rows PC..MROWS (keep pad-rank self scores finite)
            for tn in (strip[l], xr_t[l]):
                nc.sync.dma_start(
                    out=tn[PC:PC + 256, :].rearrange("(t p) d -> p t d", p=128),
                    in_=zt2[:, :].rearrange("p (t d) -> p t d", d=128))

        mmpool = es.enter_context(tc.tile_pool(name="mm", bufs=2))
        pspool = es.enter_context(tc.tile_pool(name="ps", bufs=4, space="PSUM"))
        xlpool = es.enter_context(tc.tile_pool(name="xl", bufs=3))
        hpool = es.enter_context(tc.tile_pool(name="h", bufs=2))
        apool = es.enter_context(tc.tile_pool(name="acc", bufs=2))
        spool = es.enter_context(tc.tile_pool(name="small", bufs=2))
        mpool = es.enter_context(tc.tile_pool(name="merge", bufs=2))
        epool = es.enter_context(tc.tile_pool(name="exw", bufs=2))

        mm_groups = [4] * (PC // 512) + ([(PC % 512) // 128] if PC % 512 else [])
        NL = int(os.environ.get('BASS_GAT_LAYERS', '2'))

        def mm_chunk(l, mg, xt_in=None):
            tw = mm_groups[mg]
            c0 = mg * 512
            if xt_in is None:
                assert l == 0
                xt = mmpool.tile([65, tw * 128], f16, tag="xt0", name="xt")
                nc.sync.dma_start(out=xt[:], in_=P['xT0'][0:65, c0:c0 + tw * 128])
            else:
                xt = xt_in
            sb = mmpool.tile([128, tw, 128], f32, tag="mmsb", name="sb")
            for t in range(tw):
                ps = pspool.tile([128, 128], f32, tag="mmps", name="ps")
                nc.tensor.matmul(ps[:], xt[:, t * 128:(t + 1) * 128],
                                 wcat[l][:], start=True, stop=True)
                nc.scalar.copy(sb[:, t, :], ps[:])
            # per-node score scalars a (from xl cols) and b (from xr cols)
            red = mmpool.tile([128, 4, tw], f32, tag="mmred", name="red")
            nc.vector.tensor_reduce(red[:, 0, :], sb[:, :, 0:NPOS],
                                    axis=mybir.AxisListType.X, op=mybir.AluOpType.add)
            nc.vector.tensor_reduce(red[:, 1, :], sb[:, :, NPOS:64],
                                    axis=mybir.AxisListType.X, op=mybir.AluOpType.add)
            nc.vector.tensor_reduce(red[:, 2, :], sb[:, :, 64:64 + NPOS],
                                    axis=mybir.AxisListType.X, op=mybir.AluOpType.add)
            nc.vector.tensor_reduce(red[:, 3, :], sb[:, :, 64 + NPOS:128],
                                    axis=mybir.AxisListType.X, op=mybir.AluOpType.add)
            stF = mmpool.tile([128, tw, 128], f16, tag="mmst", name="stF")
            xrF = mmpool.tile([128, tw, 128], f16, tag="mmxr", name="xrF")
            nc.scalar.copy(stF[:, :, 0:64], sb[:, :, 0:64])
            nc.scalar.copy(xrF[:, :, 0:64], sb[:, :, 64:128])
            nc.vector.tensor_sub(stF[:, :, 64], red[:, 0, :], red[:, 1, :])
            nc.vector.tensor_sub(xrF[:, :, 64], red[:, 2, :], red[:, 3, :])
            dst_xl = strip[l][c0:c0 + tw * 128, :].rearrange(
                "(t p) d -> p t d", p=128)
            dst_xr = xr_t[l][c0:c0 + tw * 128, :].rearrange(
                "(t p) d -> p t d", p=128)
            nc.scalar.dma_start(out=dst_xl, in_=stF[:])
            nc.sync.dma_start(out=dst_xr, in_=xrF[:])

        def merge_group(l, g, nd, hv):
            if hv:
                ib = mpool.tile([128, 32], i16, tag="ib", name="ib")
                nc.sync.dma_start(out=ib[:], in_=P['mB'][:, 32 * g:32 * (g + 1)])
                gb = mpool.tile([128, G, 128], f16, tag="gb", name="gb")
                nc.gpsimd.dma_gather(out_ap=gb[:], in_ap=nd_v[l][:, :], idxs_ap=ib[:],
                                     num_idxs=RG, num_idxs_reg=nireg(RG),
                                     elem_size=128, queue_num=nextq())
                sm = mpool.tile([128, G, 65], f32, tag="sm", name="sm")
                nc.vector.tensor_add(sm[:], nd[:], gb[:, :, 0:65])
            else:
                sm = mpool.tile([128, G, 65], f32, tag="sm", name="sm")
                nc.scalar.copy(sm[:], nd[:])
            rc = mpool.tile([128, G, 1], f32, tag="rc", name="rc")
            nc.vector.reciprocal(rc[:], sm[:, :, 64:65])
            hm = mpool.tile([128, G, 64], f32, tag="hm", name="hm")
            nc.vector.tensor_mul(hm[:], sm[:, :, 0:64],
                                 rc[:, :, :].to_broadcast([128, G, 64]))
            if l == 1:
                nc.vector.tensor_mul(hm[:], hm[:],
                                     rat[:, :].unsqueeze(1).to_broadcast([128, G, 64]))
            nc.vector.tensor_add(hm[:], hm[:],
                                 biasT[l][:, :].unsqueeze(1).to_broadcast([128, G, 64]))
            if l == 0:
                # write PE-transposed features straight into layer 1's moving
                # matmul operand (no hT round-trip through DRAM)
                tw = mm_groups[g] if g < len(mm_groups) else 0
                xt = mmpool.tile([65, 512], f32, tag="xt", name="xt")
                nc.vector.memset(xt[64:65, :], 1.0)
                for t in range(tw):
                    pst = pspool.tile([64, 128], f32, tag="pst", name="pst")
                    nc.tensor.transpose(pst[:], hm[:, t, :], ident[:])
                    nc.scalar.copy(xt[0:64, t * 128:(t + 1) * 128], pst[:])
                return xt
            else:
                dst_h = h2out[g * RG:(g + 1) * RG, :].rearrange(
                    "(t p) d -> p t d", p=128)
                nc.scalar.dma_start(out=dst_h, in_=hm[:])
                return None

        def slot_group(l, g, coff):
            is_v = g >= NGM
            nws = G * int(S[g].sum())
            den = apool.tile([128, G], f16, tag="den")
            nd = apool.tile([128, G, 65], f16, tag="nd")
            if not is_v:
                xr128 = spool.tile([128, G, 65], f16, tag="xr")
                nc.sync.dma_start(
                    out=xr128[:],
                    in_=xr_t[l][g * RG:(g + 1) * RG, 0:65].rearrange(
                        "(t p) d -> p t d", p=128))
                xlo = spool.tile([128, G, 65], f16, tag="xlo")
                nc.sync.dma_start(
                    out=xlo[:],
                    in_=strip[l][g * RG:(g + 1) * RG, 0:65].rearrange(
                        "(t p) d -> p t d", p=128))
                xr64 = xr128[:, :, 0:64]
                bcol = xr128[:, :, 64:65]
            else:
                vxi = spool.tile([128, 32], i16, tag="vxi")
                gv = g - NGM
                nc.sync.dma_start(out=vxi[:], in_=P['vxidx'][:, 32 * gv:32 * (gv + 1)])
                vxr = spool.tile([128, G, 128], f16, tag="vxr")
                nc.gpsimd.dma_gather(out_ap=vxr[:], in_ap=xr_t[l][:, :],
                                     idxs_ap=vxi[:], num_idxs=RG,
                                     num_idxs_reg=nireg(RG), elem_size=128,
                                     queue_num=nextq())
                xr64 = vxr[:, :, 0:64]
                bcol = vxr[:, :, 64:65]
            # idx loads ride the Scalar HWDGE queue so they are never stuck
            # behind Sync-queue waits; gathers fill one whole-group tile
            xlg = xlpool.tile([128, nws, 128], f16, tag="xl")
            spans = []
            qo = 0
            for c in range(NW):
                S_c = int(S[g][c])
                w = 8 * G * S_c
                it = spool.tile([128, w], i16, tag=f"it{c}")
                nc.sync.dma_start(out=it[:], in_=P[f'sidx{c}'][:, coff[c]:coff[c] + w])
                for q0 in range(0, G * S_c, 8):
                    qn = min(8, G * S_c - q0)
                    nc.gpsimd.dma_gather(
                        out_ap=xlg[:, qo + q0:qo + q0 + qn, :],
                        in_ap=table[l][c * WIN:(c + 1) * WIN, :],
                        idxs_ap=it[:, q0 * 8:(q0 + qn) * 8],
                        num_idxs=128 * qn, num_idxs_reg=nireg(128 * qn),
                        elem_size=128, queue_num=nextq())
                spans.append((qo, S_c))
                qo += G * S_c

            z = hpool.tile([128, nws, 64], f16, tag="z")
            sc = spool.tile([128, nws], f16, tag="sc")
            scn = spool.tile([128, nws], f16, tag="scn")
            ab = spool.tile([128, nws], f16, tag="ab")
            for (qo, S_c) in spans:
                z4 = z[:, qo:qo + G * S_c, :].rearrange("p (g s) d -> p g s d", g=G)
                xlg4 = xlg[:, qo:qo + G * S_c, 0:64].rearrange(
                    "p (g s) d -> p g s d", g=G)
                xrb = xr64.unsqueeze(2).to_broadcast([128, G, S_c, 64])
                nc.vector.tensor_add(z4, xlg4, xrb)
                sc3 = sc[:, qo:qo + G * S_c].rearrange("p (g s) -> p g s", g=G)
                scn3 = scn[:, qo:qo + G * S_c].rearrange("p (g s) -> p g s", g=G)
                nc.vector.tensor_reduce(sc3, z4[:, :, :, 0:NPOS],
                                        axis=mybir.AxisListType.X,
                                        op=mybir.AluOpType.add,
                                        apply_absolute_value=True)
                nc.vector.tensor_reduce(scn3, z4[:, :, :, NPOS:64],
                                        axis=mybir.AxisListType.X,
                                        op=mybir.AluOpType.add,
                                        apply_absolute_value=True)
                ab3 = ab[:, qo:qo + G * S_c].rearrange("p (g s) -> p g s", g=G)
                a3 = xlg[:, qo:qo + G * S_c, 64].rearrange("p (g s) -> p g s", g=G)
                nc.vector.tensor_add(ab3, a3,
                                     bcol[:, :, 0].unsqueeze(2).to_broadcast(
                                         [128, G, S_c]))
            nc.vector.tensor_sub(sc[:], sc[:], scn[:])
            # score = 0.4 * (sc + 1.5 * ab); exp via ACT scale
            nc.vector.scalar_tensor_tensor(
                sc[:], ab[:], 1.5, sc[:],
                mybir.AluOpType.mult, mybir.AluOpType.add)
            ex = spool.tile([128, nws], f16, tag="ex")
            nc.scalar.activation(ex[:], sc[:], mybir.ActivationFunctionType.Exp,
                                 scale=0.4)
            dtmp4 = spool.tile([128, NW, G], f16, tag="dtmp4")
            ntmp4 = spool.tile([128, NW, G, 64], f16, tag="ntmp4")
            for ci, (qo, S_c) in enumerate(spans):
                ex3 = ex[:, qo:qo + G * S_c].rearrange("p (g s) -> p g s", g=G)
                nc.vector.tensor_reduce(dtmp4[:, ci], ex3,
                                        axis=mybir.AxisListType.X,
                                        op=mybir.AluOpType.add)
                z4 = z[:, qo:qo + G * S_c, :].rearrange("p (g s) d -> p g s d", g=G)
                # expand ex across features on the Scalar engine so the DVE
                # multiply is fully packed, then sum slots by a halving tree
                # of packed adds (both ~2x faster than strided reduce)
                exw = epool.tile([128, G, S_c, 64], f16, tag="exw")
                nc.scalar.copy(exw[:], ex3.unsqueeze(3).to_broadcast(
                    [128, G, S_c, 64]))
                nc.vector.tensor_mul(z4, z4, exw[:])
                m = S_c
                while m > 2:
                    h = m // 2
                    nc.vector.tensor_add(z4[:, :, 0:h, :], z4[:, :, 0:h, :],
                                         z4[:, :, m - h:m, :])
                    m = m - h
                nt = ntmp4[:, ci].unsqueeze(2)
                if m == 2:
                    nc.vector.tensor_add(nt, z4[:, :, 0:1, :], z4[:, :, 1:2, :])
                else:
                    nc.vector.tensor_copy(nt, z4[:, :, 0:1, :])
            dtmp = spool.tile([128, G], f16, tag="dtmp")
            nc.vector.tensor_add(ntmp4[:, 0], ntmp4[:, 0], ntmp4[:, 1])
            nc.vector.tensor_add(ntmp4[:, 2], ntmp4[:, 2], ntmp4[:, 3])
            nc.vector.tensor_add(nd[:, :, 0:64], ntmp4[:, 0], ntmp4[:, 2])
            nc.vector.tensor_add(dtmp4[:, 0], dtmp4[:, 0], dtmp4[:, 1])
            nc.vector.tensor_add(dtmp4[:, 2], dtmp4[:, 2], dtmp4[:, 3])
            nc.vector.tensor_add(dtmp[:], dtmp4[:, 0], dtmp4[:, 2])
            if not is_v:
                # self-loop: z_self = xlo + xr, score from a_own + b
                zs = spool.tile([128, G, 64], f16, tag="zs")
                nc.vector.tensor_add(zs[:], xlo[:, :, 0:64], xr64)
                scs = spool.tile([128, 2, G], f16, tag="scs")
                nc.vector.tensor_reduce(scs[:, 0], zs[:, :, 0:NPOS],
                                        axis=mybir.AxisListType.X,
                                        op=mybir.AluOpType.add,
                                        apply_absolute_value=True)
                nc.vector.tensor_reduce(scs[:, 1], zs[:, :, NPOS:64],
                                        axis=mybir.AxisListType.X,
                                        op=mybir.AluOpType.add,
                                        apply_absolute_value=True)
                nc.vector.tensor_sub(scs[:, 0], scs[:, 0], scs[:, 1])
                abs_ = spool.tile([128, G], f16, tag="abs")
                nc.vector.tensor_add(abs_[:], xlo[:, :, 64], bcol[:, :, 0])
                nc.vector.scalar_tensor_tensor(
                    scs[:, 0], abs_[:], 1.5, scs[:, 0],
                    mybir.AluOpType.mult, mybir.AluOpType.add)
                exs = spool.tile([128, G], f16, tag="exs")
                nc.scalar.activation(exs[:], scs[:, 0],
                                     mybir.ActivationFunctionType.Exp, scale=0.4)
                nc.vector.tensor_add(den[:], exs[:], dtmp[:])
                # num += exs * z_self
                nc.vector.tensor_mul(zs[:], zs[:],
                                     exs[:, :].unsqueeze(2).to_broadcast([128, G, 64]))
                nc.vector.tensor_add(nd[:, :, 0:64], nd[:, :, 0:64], zs[:])
            else:
                nc.scalar.copy(den[:], dtmp[:])
            # z-trick correction: num -= den * xr
            corr = spool.tile([128, G, 64], f16, tag="corr")
            nc.vector.tensor_mul(corr[:], xr64,
                                 den[:, :].unsqueeze(2).to_broadcast([128, G, 64]))
            nc.vector.tensor_sub(nd[:, :, 0:64], nd[:, :, 0:64], corr[:])
            nc.scalar.copy(nd[:, :, 64:65], den[:, :].unsqueeze(2))
            if is_v:
                gv = g - NGM
                dst_nd = nd_v[l][gv * RG:(gv + 1) * RG, 0:65].rearrange(
                    "(t p) d -> p t d", p=128)
                nc.scalar.dma_start(out=dst_nd, in_=nd[:])
            return nd

        for l in range(NL):
            # ---- matmul phase (layer 0 only; layer 1 fused into merges) ----
            if l == 0:
                for mg in range(len(mm_groups)):
                    mm_chunk(l, mg)
            # poison row for padded slots (window-local row NS of even strip)
            nc.sync.dma_start(out=strip[l][NS:NS + 1, :], in_=poison[:])
            # ---- all-gather the table ----
            nc.gpsimd.collective_compute(
                "AllGather", mybir.AluOpType.bypass,
                replica_groups=[list(range(M))],
                ins=[strip[l][0:PC, :]], outs=[table[l][:, :]])

            # ---- slot offsets ----
            coff = [0] * NW
            coff_at = {}
            for g in range(NG):
                coff_at[g] = list(coff)
                for c in range(NW):
                    coff[c] += 8 * G * int(S[g][c])
            # ---- virtual groups first, then main + fused merge (+ next mm) --
            for g in range(NGM, NG):
                slot_group(l, g, coff_at[g])
            for g in range(NGM):
                nd = slot_group(l, g, coff_at[g])
                xt = merge_group(l, g, nd, HV[g])
                if l == 0 and NL > 1:
                    mm_chunk(1, g, xt_in=xt)

    nc.compile()
    return nc


# ----------------------------------------------------------------------
# entry point
# ----------------------------------------------------------------------

def kernel(**inputs):
    x = np.asarray(inputs['x'], np.float32)
    ei = np.asarray(inputs['edge_index'])
    W_l = np.asarray(inputs['W_l'], np.float64)
    b_l = np.asarray(inputs['b_l'], np.float64)
    W_r = np.asarray(inputs['W_r'], np.float64)
    b_r = np.asarray(inputs['b_r'], np.float64)
    att = np.asarray(inputs['att'], np.float64)
    bias = np.asarray(inputs['bias'], np.float64)

    T = preprocess(x, ei)

    # fold |att| into the weights; permute features so att>0 comes first
    Pm = np.concatenate([np.where(att > 0)[0], np.where(att <= 0)[0]])
    NPOS = int((att > 0).sum())
    aab = np.abs(att[Pm])
    aab[aab == 0] = 1.0

    nc = build_program(T['S'], T['NGV'], NPOS, T['hv'])

    def wcat_of(Wl, bl, Wr, br):
        Wc = np.zeros((65, 128), np.float32)
        Wc[:64, 0:64] = Wl
        Wc[64, 0:64] = bl
        Wc[:64, 64:128] = Wr
        Wc[64, 64:128] = br
        return Wc

    Wl1 = W_l[:, Pm] * aab; bl1 = b_l[Pm] * aab
    Wr1 = W_r[:, Pm] * aab; br1 = b_r[Pm] * aab
    Wl2 = (W_l[Pm][:, Pm] * aab) / aab[:, None]
    Wr2 = (W_r[Pm][:, Pm] * aab) / aab[:, None]
    Wcat0 = wcat_of(Wl1, bl1, Wr1, br1)
    Wcat1 = wcat_of(Wl2, bl1, Wr2, br1)
    bias0 = np.tile((bias[Pm] * aab)[None, :], (128, 1)).astype(np.float32)
    bias1 = np.tile(bias[Pm][None, :], (128, 1)).astype(np.float32)
    rat = np.tile((1.0 / aab)[None, :], (128, 1)).astype(np.float32)
    ident = np.eye(128, dtype=np.float32)
    poison = np.zeros((1, 128), np.float16)
    poison[0, 64] = POISON_A

    in_maps = []
    for m in range(M):
        xT0 = np.zeros((65, PC), np.float16)
        xT0[0:64, 0:NS] = x[T['node_order'][m]].T
        xT0[64, :] = 1.0
        pcm = T['percore'][m]
        im = dict(xT0=xT0, Wcat0=Wcat0.astype(np.float16), Wcat1=Wcat1, bias0=bias0, bias1=bias1,
                  rat=rat, ident=ident, poison=poison,
                  vxidx=pcm['vxidx'], mB=pcm['mB'])
        for c in range(NW):
            im[f'sidx{c}'] = pcm['sidx'][c]
        in_maps.append(im)

    if os.environ.get('BASS_GAT_SIM'):
        from concourse import bass_interp
        sim = bass_interp.MultiCoreSim(nc, M)
        for m in range(M):
            for k, v in in_maps[m].items():
                sim.cores[m].tensor(k)[:] = v
        sim.simulate()
        results = [{'h2': sim.cores[m].mem_tensor('h2')} for m in range(M)]
    else:
        from concourse.bass_utils import run_bass_kernel_spmd
        trace = bool(os.environ.get('BASS_GAT_TRACE'))
        res = run_bass_kernel_spmd(nc, in_maps, list(range(M)), trace=trace)
        if trace:
            print(f"[hw] exec_time_ns: {res.exec_time_ns}")
            print(f"HW exec time: {res.exec_time_ns} ns")
        results = res.results

    h2 = np.zeros((N, D), np.float32)
    for m in range(M):
        h2[np.ix_(T['node_order'][m], Pm)] = results[m]['h2'][:NS]
    return x + h2


# revision 21
# speedup vs baseline: 1.0689x; 1.0689x over previous
"""GATv2 x2 + residual on 8 TRN2 NeuronCores (Bass/Tile).

Strategy (self-contained; N=100000, D=64, E=1700000):

- Nodes are assigned to the 8 cores by a greedy balancer (12500 each),
  then ordered per-core by descending in-degree ("device order"). All
  device tables (strips, nd accumulators, output) use this order, so
  per-destination loads are plain sequential DMAs instead of gathers.
- |att| is folded into the weights and features are permuted so att>0
  features come first. The per-edge score uses the exact identity
  lrelu_0.2(z) = 0.6 z + 0.4|z|:
      score = 0.6 (a_src + b_dst) + 0.4 (sum_pos|z| - sum_neg|z|)
  where a = sum_pos xl - sum_neg xl and b likewise for xr are per-node
  scalars computed once in the matmul phase and carried as column 64 of
  the fp16 strips. This removes the per-slot leaky-relu pass entirely;
  the |.| sums use tensor_reduce(apply_absolute_value=True).
- All per-edge tensors are fp16 (2x DVE throughput on packed ops); the
  weighted message sum uses the z-trick
      num = sum ex*z - den*xr            (z = xl + xr)
  so the gathered xl tile is only read once (by the z add).
- Tables are fp16 with 128-wide rows ([xl(64) | a | pad]): one 256B
  gather descriptor per edge, same as f32/64-wide. Gathers cycle over 4
  SWDGE queues with a 64KB descriptor ring.
- Self-loops never enter the edge streams; slot padding gathers a
  poison row (a = -1e4) whose score underflows exp to exactly 0.
- Per-row softmax over destination-major slots [128 dst x slots];
  heavy rows overflow to per-core compacted virtual rows (nd_v).
  Virtual groups run FIRST; each main group is immediately followed by
  its merge (consuming the nd accumulator tile directly from SBUF) and,
  in layer 0, by the layer-1 matmul chunk for the same columns, whose
  moving operand the merge assembles in SBUF from its PE transposes --
  no nd/hT DRAM round-trips and no serial merge/matmul phases. Merges
  of groups with no virtual rows on any core skip the nd_v gather.
- exp(score) is expanded across the 64 features by the Scalar engine so
  the DVE weight multiply is fully packed; slot sums use a halving tree
  of packed adds instead of strided reduces. Layer-0 matmuls run in
  fp16 (1 PE pass). Layer 2's merge writes the final features; the
  host adds the residual.
"""
import os
import numpy as np

N = 100000
D = 64
M = 8
NS = 12500              # real nodes per core
PC = 12544              # padded strip rows (98 * 128)
NW = 4                  # gather windows (core pairs)
WIN = 2 * PC            # rows per window
NEG = 0.2
CAPQ = 0.75             # slot cap quantile within a group
G = 4                   # 128-row blocks per group
RG = 128 * G            # rows per group (512)
NGM = 25                # main groups (25*512 = 12800 >= NS)
MROWS = NGM * RG        # 12800
TBL = M * PC            # table rows (100352)
POISON_A = -10000.0
PIDX = NS               # window-local poison row (even strip, row NS)


# ----------------------------------------------------------------------
# host preprocessing
# ----------------------------------------------------------------------

def _assign_cores(src, dst):
    order = np.argsort(src, kind='stable')
    d_sorted = dst[order]
    starts = np.searchsorted(src[order], np.arange(N + 1))
    core = np.full(N, -1, np.int32)
    quota = np.full(M, NS, np.int64)
    cnt = np.zeros((N, NW), np.float32)
    outdeg = starts[1:] - starts[:-1]
    proc = np.argsort(-outdeg, kind='stable')
    pair_edges = np.zeros(NW, np.float64)
    for v in proc:
        ds = d_sorted[starts[v]:starts[v + 1]]
        costs = cnt[ds].sum(axis=0) if len(ds) else np.zeros(NW, np.float32)
        costs = costs + 1e-7 * pair_edges
        best = None
        for p in np.argsort(costs, kind='stable'):
            if quota[2 * p] > 0 or quota[2 * p + 1] > 0:
                best = int(p)
                break
        c0, c1 = 2 * best, 2 * best + 1
        c = c0 if quota[c0] >= quota[c1] else c1
        core[v] = c
        quota[c] -= 1
        if len(ds):
            cnt[ds, best] += 1.0
            pair_edges[best] += len(ds)
    return core


def _wrap_idx(flat):
    """dma_gather index layout: [128, n/16] int16, 16-wrapped, 8x replicated."""
    n = len(flat)
    assert n % 16 == 0
    w = flat.reshape(n // 16, 16).T
    return np.ascontiguousarray(np.tile(w, (8, 1)), dtype=np.int16)


def preprocess(x, edge_index):
    src, dst = np.asarray(edge_index[0]), np.asarray(edge_index[1])
    nonself = src != dst
    src, dst = src[nonself].astype(np.int64), dst[nonself].astype(np.int64)
    core = _assign_cores(np.asarray(edge_index[0]), np.asarray(edge_index[1]))
    win_of = core // 2

    # provisional per-core rank (any order), then degree-sort -> device order
    node_order = np.zeros((M, NS), np.int64)
    for m in range(M):
        ids = np.where(core == m)[0]
        node_order[m] = ids
    # per-node non-self in-degree
    indeg = np.bincount(dst, minlength=N)
    for m in range(M):
        o = np.argsort(-indeg[node_order[m]], kind='stable')
        node_order[m] = node_order[m][o]
    rank = np.zeros(N, np.int64)
    for m in range(M):
        rank[node_order[m]] = np.arange(NS)
    twl = (core % 2).astype(np.int64) * PC + rank       # window-local table row

    # per-core edge lists grouped by (dst rank, window)
    svals, bounds = [], []
    for m in range(M):
        em = np.where(core[dst] == m)[0]
        es, ed = src[em], dst[em]
        key = rank[ed] * NW + win_of[es]
        # ascending src within each (rank, window) segment: gather columns
        # then cluster into narrow table ranges (better HBM locality)
        ko = np.lexsort((twl[es], key))
        svals.append(twl[es[ko]])
        bounds.append(np.searchsorted(key[ko], np.arange(NS * NW + 1)))

    cnts = np.zeros((M, NS, NW), np.int64)
    for m in range(M):
        b = bounds[m]
        cnts[m] = (b[1:] - b[:-1]).reshape(NS, NW)

    # shared main-group slot caps (device order = degree order)
    S_main = np.zeros((NGM, NW), np.int32)
    for g in range(NGM):
        caps = np.zeros((M, NW), np.int32)
        for m in range(M):
            rows = cnts[m][g * RG:(g + 1) * RG]
            if len(rows) == 0:
                continue
            caps[m] = np.ceil(np.quantile(rows, CAPQ, axis=0)).astype(np.int32)
        S_main[g] = caps.max(axis=0)
    S_main = np.maximum(S_main, 1)

    # overflow -> per-core compacted virtual rows
    gidx = np.minimum(np.arange(NS) // RG, NGM - 1)
    ov = np.maximum(cnts - S_main[gidx][None, :, :], 0)   # [M, NS, NW]
    virt = []
    for m in range(M):
        v = np.where(ov[m].sum(axis=1) > 0)[0]
        virt.append(v[np.argsort(-ov[m][v].sum(axis=1), kind='stable')])
    NV = max(len(v) for v in virt)
    NGV = (NV + RG - 1) // RG
    S_virt = np.zeros((NGV, NW), np.int32)
    for g in range(NGV):
        mx = np.zeros(NW, np.int32)
        for m in range(M):
            v = virt[m][g * RG:(g + 1) * RG]
            if len(v):
                mx = np.maximum(mx, ov[m][v].max(axis=0))
        S_virt[g] = np.maximum(mx, 1)

    NG = NGM + NGV
    S = np.concatenate([S_main, S_virt], axis=0)         # [NG, NW]
    R = NG * RG
    ZR = R                                               # zero-row id

    hv_glob = np.zeros(NGM, bool)
    percore = []
    for m in range(M):
        b = bounds[m]
        sv = svals[m]
        vr = virt[m]
        virtrow = np.full(NS, NGV * RG, np.int64)  # default: zero row (virt-local)
        sidx = [[] for _ in range(NW)]
        vxidx = []
        for g in range(NG):
            is_v = g >= NGM
            for c in range(NW):
                S_c = int(S[g, c])
                flat = np.full(128 * G * S_c, PIDX, np.int16)
                for gg in range(G):
                    for p in range(128):
                        i = gg * 128 + p
                        if not is_v:
                            r = g * RG + i
                            if r >= NS:
                                continue
                        else:
                            gi = (g - NGM) * RG + i
                            if gi >= len(vr):
                                continue
                            r = vr[gi]
                            if c == 0:
                                virtrow[r] = (g - NGM) * RG + i
                        lo, hi = b[r * NW + c], b[r * NW + c + 1]
                        if is_v:
                            lo = lo + int(S_main[min(r // RG, NGM - 1), c])
                        seg = sv[lo:min(hi, lo + S_c)]
                        for s_i, v_ in enumerate(seg):
                            flat[(gg * S_c + s_i) * 128 + p] = v_
                sidx[c].append(_wrap_idx(flat))
            if is_v:
                vx = np.zeros(RG, np.int16)
                gi0 = (g - NGM) * RG
                for i in range(RG):
                    gi = gi0 + i
                    vx[i] = vr[gi] if gi < len(vr) else NS  # pad: xr row NS
                vxidx.append(_wrap_idx(vx))
        mB = np.full(MROWS, NGV * RG, np.int16)
        mB[:NS] = virtrow
        vranks = np.where(virtrow != NGV * RG)[0]
        hv_glob[np.minimum(vranks // RG, NGM - 1)] = True
        percore.append(dict(
            sidx=[np.concatenate(s, axis=1) for s in sidx],
            vxidx=(np.concatenate(vxidx, axis=1) if NGV else
                   np.zeros((128, 32), np.int16)),
            mB=np.concatenate([_wrap_idx(mB[g * RG:(g + 1) * RG])
                               for g in range(NGM)], axis=1),
        ))
    return dict(core=core, node_order=node_order, S=S, NGV=NGV, NG=NG,
                R=R, percore=percore, hv=hv_glob.tolist())


# ----------------------------------------------------------------------
# device program
# ----------------------------------------------------------------------

def build_program(S, NGV, NPOS, HV=None):
    from concourse import bass, mybir, tile
    from concourse import bacc
    f32 = mybir.dt.float32
    f16 = mybir.dt.float16
    i16 = mybir.dt.int16
    NG = NGM + NGV
    R = NG * RG
    CC = [sum(8 * G * int(S[g][c]) for g in range(NG)) for c in range(NW)]
    assert 0 < NPOS < 64
    if HV is None:
        HV = [True] * NGM

    nc = bacc.Bacc(num_swdge_queues=4, dynamic_dma_scratch_size=64512)
    P = {}
    P['xT0'] = nc.declare_dram_parameter("xT0", [65, PC], f16, isOutput=False)
    P['Wcat0'] = nc.declare_dram_parameter("Wcat0", [65, 128], f16, isOutput=False)
    P['Wcat1'] = nc.declare_dram_parameter("Wcat1", [65, 128], f32, isOutput=False)
    P['bias0'] = nc.declare_dram_parameter("bias0", [128, 64], f32, isOutput=False)
    P['bias1'] = nc.declare_dram_parameter("bias1", [128, 64], f32, isOutput=False)
    P['rat'] = nc.declare_dram_parameter("rat", [128, 64], f32, isOutput=False)
    P['ident'] = nc.declare_dram_parameter("ident", [128, 128], f32, isOutput=False)
    P['poison'] = nc.declare_dram_parameter("poison", [1, 128], f16, isOutput=False)
    for c in range(NW):
        P[f'sidx{c}'] = nc.declare_dram_parameter(f"sidx{c}", [128, CC[c]], i16, isOutput=False)
    P['vxidx'] = nc.declare_dram_parameter("vxidx", [128, max(32 * NGV, 32)], i16, isOutput=False)
    P['mB'] = nc.declare_dram_parameter("mB", [128, 32 * NGM], i16, isOutput=False)
    h2out = nc.declare_dram_parameter("h2", [MROWS, 64], f32, isOutput=True)

    strip = [nc.dram_tensor(f"strip{l}", [MROWS, 128], f16) for l in range(2)]
    xr_t = [nc.dram_tensor(f"xr{l}", [MROWS, 128], f16) for l in range(2)]
    table = [nc.dram_tensor(f"table{l}", [TBL, 128], f16, addr_space="Shared")
             for l in range(2)]
    nd_v = [nc.dram_tensor(f"ndv{l}", [NGV * RG + 128, 128], f16) for l in range(2)]

    from contextlib import ExitStack
    _regs = {}

    def nireg(v):
        if v not in _regs:
            r = nc.gpsimd.alloc_register(f"ni{v}")
            nc.gpsimd.reg_mov(r, v)
            _regs[v] = r
        return _regs[v]

    _qctr = [0]

    def nextq():
        q = _qctr[0] % 4
        _qctr[0] += 1
        return q

    with tile.TileContext(nc) as tc, ExitStack() as es, \
            nc.allow_low_precision(reason="fp16 softmax accumulators"):
        cpool = es.enter_context(tc.tile_pool(name="const", bufs=1))
        wcat = [cpool.tile([65, 128], f16 if i == 0 else f32, name=f"wcat{i}")
                for i in range(2)]
        biasT = [cpool.tile([128, 64], f32, name=f"biasT{i}") for i in range(2)]
        rat = cpool.tile([128, 64], f32)
        ident = cpool.tile([128, 128], f32)
        poison = cpool.tile([1, 128], f16)
        for l in range(2):
            nc.sync.dma_start(out=wcat[l][:], in_=P[f'Wcat{l}'][:, :])
            nc.sync.dma_start(out=biasT[l][:], in_=P[f'bias{l}'][:, :])
        nc.sync.dma_start(out=rat[:], in_=P['rat'][:, :])
        nc.sync.dma_start(out=ident[:], in_=P['ident'][:, :])
        nc.sync.dma_start(out=poison[:], in_=P['poison'][:, :])
        # zero-rows of nd tables (fp16); ones-row of hT
        zt = cpool.tile([128, 128], f16)
        nc.vector.memset(zt[:], 0.0)
        zt2 = cpool.tile([128, 256], f16)
        nc.vector.memset(zt2[:], 0.0)
        for l in range(2):
            nc.sync.dma_start(out=nd_v[l][NGV * RG:NGV * RG + 128, :], in_=zt[:])
            # zero tail rows PC..MROWS (keep pad-rank self scores finite)
            for tn in (strip[l], xr_t[l]):
                nc.sync.dma_start(
                    out=tn[PC:PC + 256, :].rearrange("(t p) d -> p t d", p=128),
                    in_=zt2[:, :].rearrange("p (t d) -> p t d", d=128))

        mmpool = es.enter_context(tc.tile_pool(name="mm", bufs=2))
        pspool = es.enter_context(tc.tile_pool(name="ps", bufs=4, space="PSUM"))
        xlpool = es.enter_context(tc.tile_pool(name="xl", bufs=3))
        hpool = es.enter_context(tc.tile_pool(name="h", bufs=2))
        apool = es.enter_context(tc.tile_pool(name="acc", bufs=2))
        spool = es.enter_context(tc.tile_pool(name="small", bufs=2))
        mpool = es.enter_context(tc.tile_pool(name="merge", bufs=2))
        epool = es.enter_context(tc.tile_pool(name="exw", bufs=2))

        mm_groups = [4] * (PC // 512) + ([(PC % 512) // 128] if PC % 512 else [])
        NL = int(os.environ.get('BASS_GAT_LAYERS', '2'))

        def mm_chunk(l, mg, xt_in=None):
            tw = mm_groups[mg]
            c0 = mg * 512
            if xt_in is None:
                assert l == 0
                xt = mmpool.tile([65, tw * 128], f16, tag="xt0", name="xt")
                nc.sync.dma_start(out=xt[:], in_=P['xT0'][0:65, c0:c0 + tw * 128])
            else:
                xt = xt_in
            sb = mmpool.tile([128, tw, 128], f32, tag="mmsb", name="sb")
            for t in range(tw):
                ps = pspool.tile([128, 128], f32, tag="mmps", name="ps")
                nc.tensor.matmul(ps[:], xt[:, t * 128:(t + 1) * 128],
                                 wcat[l][:], start=True, stop=True)
                nc.scalar.copy(sb[:, t, :], ps[:])
            # per-node score scalars a (from xl cols) and b (from xr cols)
            red = mmpool.tile([128, 4, tw], f32, tag="mmred", name="red")
            nc.vector.tensor_reduce(red[:, 0, :], sb[:, :, 0:NPOS],
                                    axis=mybir.AxisListType.X, op=mybir.AluOpType.add)
            nc.vector.tensor_reduce(red[:, 1, :], sb[:, :, NPOS:64],
                                    axis=mybir.AxisListType.X, op=mybir.AluOpType.add)
            nc.vector.tensor_reduce(red[:, 2, :], sb[:, :, 64:64 + NPOS],
                                    axis=mybir.AxisListType.X, op=mybir.AluOpType.add)
            nc.vector.tensor_reduce(red[:, 3, :], sb[:, :, 64 + NPOS:128],
                                    axis=mybir.AxisListType.X, op=mybir.AluOpType.add)
            stF = mmpool.tile([128, tw, 128], f16, tag="mmst", name="stF")
            xrF = mmpool.tile([128, tw, 128], f16, tag="mmxr", name="xrF")
            nc.scalar.copy(stF[:, :, 0:64], sb[:, :, 0:64])
            nc.scalar.copy(xrF[:, :, 0:64], sb[:, :, 64:128])
            nc.vector.tensor_sub(stF[:, :, 64], red[:, 0, :], red[:, 1, :])
            nc.vector.tensor_sub(xrF[:, :, 64], red[:, 2, :], red[:, 3, :])
            dst_xl = strip[l][c0:c0 + tw * 128, :].rearrange(
                "(t p) d -> p t d", p=128)
            dst_xr = xr_t[l][c0:c0 + tw * 128, :].rearrange(
                "(t p) d -> p t d", p=128)
            nc.scalar.dma_start(out=dst_xl, in_=stF[:])
            nc.sync.dma_start(out=dst_xr, in_=xrF[:])

        def merge_group(l, g, nd, hv):
            if hv:
                ib = mpool.tile([128, 32], i16, tag="ib", name="ib")
                nc.sync.dma_start(out=ib[:], in_=P['mB'][:, 32 * g:32 * (g + 1)])
                gb = mpool.tile([128, G, 128], f16, tag="gb", name="gb")
                nc.gpsimd.dma_gather(out_ap=gb[:], in_ap=nd_v[l][:, :], idxs_ap=ib[:],
                                     num_idxs=RG, num_idxs_reg=nireg(RG),
                                     elem_size=128, queue_num=nextq())
                sm = mpool.tile([128, G, 65], f32, tag="sm", name="sm")
                nc.vector.tensor_add(sm[:], nd[:], gb[:, :, 0:65])
            else:
                sm = mpool.tile([128, G, 65], f32, tag="sm", name="sm")
                nc.scalar.copy(sm[:], nd[:])
            rc = mpool.tile([128, G, 1], f32, tag="rc", name="rc")
            nc.vector.reciprocal(rc[:], sm[:, :, 64:65])
            hm = mpool.tile([128, G, 64], f32, tag="hm", name="hm")
            nc.vector.tensor_mul(hm[:], sm[:, :, 0:64],
                                 rc[:, :, :].to_broadcast([128, G, 64]))
            if l == 1:
                nc.vector.tensor_mul(hm[:], hm[:],
                                     rat[:, :].unsqueeze(1).to_broadcast([128, G, 64]))
            nc.vector.tensor_add(hm[:], hm[:],
                                 biasT[l][:, :].unsqueeze(1).to_broadcast([128, G, 64]))
            if l == 0:
                # write PE-transposed features straight into layer 1's moving
                # matmul operand (no hT round-trip through DRAM)
                tw = mm_groups[g] if g < len(mm_groups) else 0
                xt = mmpool.tile([65, 512], f32, tag="xt", name="xt")
                nc.vector.memset(xt[64:65, :], 1.0)
                for t in range(tw):
                    pst = pspool.tile([64, 128], f32, tag="pst", name="pst")
                    nc.tensor.transpose(pst[:], hm[:, t, :], ident[:])
                    nc.scalar.copy(xt[0:64, t * 128:(t + 1) * 128], pst[:])
                return xt
            else:
                dst_h = h2out[g * RG:(g + 1) * RG, :].rearrange(
                    "(t p) d -> p t d", p=128)
                nc.scalar.dma_start(out=dst_h, in_=hm[:])
                return None

        def slot_group(l, g, coff):
            is_v = g >= NGM
            nws = G * int(S[g].sum())
            den = apool.tile([128, G], f16, tag="den")
            nd = apool.tile([128, G, 65], f16, tag="nd")
            if not is_v:
                xr128 = spool.tile([128, G, 65], f16, tag="xr")
                nc.sync.dma_start(
                    out=xr128[:],
                    in_=xr_t[l][g * RG:(g + 1) * RG, 0:65].rearrange(
                        "(t p) d -> p t d", p=128))
                xlo = spool.tile([128, G, 65], f16, tag="xlo")
                nc.sync.dma_start(
                    out=xlo[:],
                    in_=strip[l][g * RG:(g + 1) * RG, 0:65].rearrange(
                        "(t p) d -> p t d", p=128))
                xr64 = xr128[:, :, 0:64]
                bcol = xr128[:, :, 64:65]
            else:
                vxi = spool.tile([128, 32], i16, tag="vxi")
                gv = g - NGM
                nc.sync.dma_start(out=vxi[:], in_=P['vxidx'][:, 32 * gv:32 * (gv + 1)])
                vxr = spool.tile([128, G, 128], f16, tag="vxr")
                nc.gpsimd.dma_gather(out_ap=vxr[:], in_ap=xr_t[l][:, :],
                                     idxs_ap=vxi[:], num_idxs=RG,
                                     num_idxs_reg=nireg(RG), elem_size=128,
                                     queue_num=nextq())
                xr64 = vxr[:, :, 0:64]
                bcol = vxr[:, :, 64:65]
            # idx loads ride the Scalar HWDGE queue so they are never stuck
            # behind Sync-queue waits; gathers fill one whole-group tile
            xlg = xlpool.tile([128, nws, 128], f16, tag="xl")
            spans = []
            qo = 0
            for c in range(NW):
                S_c = int(S[g][c])
                w = 8 * G * S_c
                it = spool.tile([128, w], i16, tag=f"it{c}")
                nc.sync.dma_start(out=it[:], in_=P[f'sidx{c}'][:, coff[c]:coff[c] + w])
                for q0 in range(0, G * S_c, 8):
                    qn = min(8, G * S_c - q0)
                    nc.gpsimd.dma_gather(
                        out_ap=xlg[:, qo + q0:qo + q0 + qn, :],
                        in_ap=table[l][c * WIN:(c + 1) * WIN, :],
                        idxs_ap=it[:, q0 * 8:(q0 + qn) * 8],
                        num_idxs=128 * qn, num_idxs_reg=nireg(128 * qn),
                        elem_size=128, queue_num=nextq())
                spans.append((qo, S_c))
                qo += G * S_c

            z = hpool.tile([128, nws, 64], f16, tag="z")
            sc = spool.tile([128, nws], f16, tag="sc")
            scn = spool.tile([128, nws], f16, tag="scn")
            ab = spool.tile([128, nws], f16, tag="ab")
            for (qo, S_c) in spans:
                z4 = z[:, qo:qo + G * S_c, :].rearrange("p (g s) d -> p g s d", g=G)
                xlg4 = xlg[:, qo:qo + G * S_c, 0:64].rearrange(
                    "p (g s) d -> p g s d", g=G)
                xrb = xr64.unsqueeze(2).to_broadcast([128, G, S_c, 64])
                nc.vector.tensor_add(z4, xlg4, xrb)
                sc3 = sc[:, qo:qo + G * S_c].rearrange("p (g s) -> p g s", g=G)
                scn3 = scn[:, qo:qo + G * S_c].rearrange("p (g s) -> p g s", g=G)
                nc.vector.tensor_reduce(sc3, z4[:, :, :, 0:NPOS],
                                        axis=mybir.AxisListType.X,
                                        op=mybir.AluOpType.add,
                                        apply_absolute_value=True)
                nc.vector.tensor_reduce(scn3, z4[:, :, :, NPOS:64],
                                        axis=mybir.AxisListType.X,
                                        op=mybir.AluOpType.add,
                                        apply_absolute_value=True)
                ab3 = ab[:, qo:qo + G * S_c].rearrange("p (g s) -> p g s", g=G)
                a3 = xlg[:, qo:qo + G * S_c, 64].rearrange("p (g s) -> p g s", g=G)
                nc.vector.tensor_add(ab3, a3,
                                     bcol[:, :, 0].unsqueeze(2).to_broadcast(
                                         [128, G, S_c]))
            nc.vector.tensor_sub(sc[:], sc[:], scn[:])
            # score = 0.4 * (sc + 1.5 * ab); exp via ACT scale
            nc.vector.scalar_tensor_tensor(
                sc[:], ab[:], 1.5, sc[:],
                mybir.AluOpType.mult, mybir.AluOpType.add)
            ex = spool.tile([128, nws], f16, tag="ex")
            nc.scalar.activation(ex[:], sc[:], mybir.ActivationFunctionType.Exp,
                                 scale=0.4)
            dtmp4 = spool.tile([128, NW, G], f16, tag="dtmp4")
            ntmp4 = spool.tile([128, NW, G, 64], f16, tag="ntmp4")
            for ci, (qo, S_c) in enumerate(spans):
                ex3 = ex[:, qo:qo + G * S_c].rearrange("p (g s) -> p g s", g=G)
                nc.vector.tensor_reduce(dtmp4[:, ci], ex3,
                                        axis=mybir.AxisListType.X,
                                        op=mybir.AluOpType.add)
                z4 = z[:, qo:qo + G * S_c, :].rearrange("p (g s) d -> p g s d", g=G)
                # expand ex across features on the Scalar engine so the DVE
                # multiply is fully packed, then sum slots by a halving tree
                # of packed adds (both ~2x faster than strided reduce)
                exw = epool.tile([128, G, S_c, 64], f16, tag="exw")
                nc.scalar.copy(exw[:], ex3.unsqueeze(3).to_broadcast(
                    [128, G, S_c, 64]))
                nc.vector.tensor_mul(z4, z4, exw[:])
                m = S_c
                while m > 2:
                    h = m // 2
                    nc.vector.tensor_add(z4[:, :, 0:h, :], z4[:, :, 0:h, :],
                                         z4[:, :, m - h:m, :])
                    m = m - h
                nt = ntmp4[:, ci].unsqueeze(2)
                if m == 2:
                    nc.vector.tensor_add(nt, z4[:, :, 0:1, :], z4[:, :, 1:2, :])
                else:
                    nc.vector.tensor_copy(nt, z4[:, :, 0:1, :])
            dtmp = spool.tile([128, G], f16, tag="dtmp")
            nc.vector.tensor_add(ntmp4[:, 0], ntmp4[:, 0], ntmp4[:, 1])
            nc.vector.tensor_add(ntmp4[:, 2], ntmp4[:, 2], ntmp4[:, 3])
            nc.vector.tensor_add(nd[:, :, 0:64], ntmp4[:, 0], ntmp4[:, 2])
            nc.vector.tensor_add(dtmp4[:, 0], dtmp4[:, 0], dtmp4[:, 1])
            nc.vector.tensor_add(dtmp4[:, 2], dtmp4[:, 2], dtmp4[:, 3])
            nc.vector.tensor_add(dtmp[:], dtmp4[:, 0], dtmp4[:, 2])
            if not is_v:
                # self-loop: z_self = xlo + xr, score from a_own + b
                zs = spool.tile([128, G, 64], f16, tag="zs")
                nc.vector.tensor_add(zs[:], xlo[:, :, 0:64], xr64)
                scs = spool.tile([128, 2, G], f16, tag="scs")
                nc.vector.tensor_reduce(scs[:, 0], zs[:, :, 0:NPOS],
                                        axis=mybir.AxisListType.X,
                                        op=mybir.AluOpType.add,
                                        apply_absolute_value=True)
                nc.vector.tensor_reduce(scs[:, 1], zs[:, :, NPOS:64],
                                        axis=mybir.AxisListType.X,
                                        op=mybir.AluOpType.add,
                                        apply_absolute_value=True)
                nc.vector.tensor_sub(scs[:, 0], scs[:, 0], scs[:, 1])
                abs_ = spool.tile([128, G], f16, tag="abs")
                nc.vector.tensor_add(abs_[:], xlo[:, :, 64], bcol[:, :, 0])
                nc.vector.scalar_tensor_tensor(
                    scs[:, 0], abs_[:], 1.5, scs[:, 0],
                    mybir.AluOpType.mult, mybir.AluOpType.add)
                exs = spool.tile([128, G], f16, tag="exs")
                nc.scalar.activation(exs[:], scs[:, 0],
                                     mybir.ActivationFunctionType.Exp, scale=0.4)
                nc.vector.tensor_add(den[:], exs[:], dtmp[:])
                # num += exs * z_self
                nc.vector.tensor_mul(zs[:], zs[:],
                                     exs[:, :].unsqueeze(2).to_broadcast([128, G, 64]))
                nc.vector.tensor_add(nd[:, :, 0:64], nd[:, :, 0:64], zs[:])
            else:
                nc.scalar.copy(den[:], dtmp[:])
            # z-trick correction: num -= den * xr
            corr = spool.tile([128, G, 64], f16, tag="corr")
            nc.vector.tensor_mul(corr[:], xr64,
                                 den[:, :].unsqueeze(2).to_broadcast([128, G, 64]))
            nc.vector.tensor_sub(nd[:, :, 0:64], nd[:, :, 0:64], corr[:])
            nc.scalar.copy(nd[:, :, 64:65], den[:, :].unsqueeze(2))
            if is_v:
                gv = g - NGM
                dst_nd = nd_v[l][gv * RG:(gv + 1) * RG, 0:65].rearrange(
                    "(t p) d -> p t d", p=128)
                nc.scalar.dma_start(out=dst_nd, in_=nd[:])
            return nd

        for l in range(NL):
            # ---- matmul phase (layer 0 only; layer 1 fused into merges) ----
            if l == 0:
                for mg in range(len(mm_groups)):
                    mm_chunk(l, mg)
            # poison row for padded slots (window-local row NS of even strip)
            nc.sync.dma_start(out=strip[l][NS:NS + 1, :], in_=poison[:])
            # ---- all-gather the table ----
            nc.gpsimd.collective_compute(
                "AllGather", mybir.AluOpType.bypass,
                replica_groups=[list(range(M))],
                ins=[strip[l][0:PC, :]], outs=[table[l][:, :]])

            # ---- slot offsets ----
            coff = [0] * NW
            coff_at = {}
            for g in range(NG):
                coff_at[g] = list(coff)
                for c in range(NW):
                    coff[c] += 8 * G * int(S[g][c])
            # ---- virtual groups first, then main + fused merge (+ next mm) --
            for g in range(NGM, NG):
                slot_group(l, g, coff_at[g])
            for g in range(NGM):
                nd = slot_group(l, g, coff_at[g])
                xt = merge_group(l, g, nd, HV[g])
                if l == 0 and NL > 1:
                    mm_chunk(1, g, xt_in=xt)

    nc.compile()
    return nc


# ----------------------------------------------------------------------
# entry point
# ----------------------------------------------------------------------

def kernel(**inputs):
    x = np.asarray(inputs['x'], np.float32)
    ei = np.asarray(inputs['edge_index'])
    W_l = np.asarray(inputs['W_l'], np.float64)
    b_l = np.asarray(inputs['b_l'], np.float64)
    W_r = np.asarray(inputs['W_r'], np.float64)
    b_r = np.asarray(inputs['b_r'], np.float64)
    att = np.asarray(inputs['att'], np.float64)
    bias = np.asarray(inputs['bias'], np.float64)

    T = preprocess(x, ei)

    # fold |att| into the weights; permute features so att>0 comes first
    Pm = np.concatenate([np.where(att > 0)[0], np.where(att <= 0)[0]])
    NPOS = int((att > 0).sum())
    aab = np.abs(att[Pm])
    aab[aab == 0] = 1.0

    nc = build_program(T['S'], T['NGV'], NPOS, T['hv'])

    def wcat_of(Wl, bl, Wr, br):
        Wc = np.zeros((65, 128), np.float32)
        Wc[:64, 0:64] = Wl
        Wc[64, 0:64] = bl
        Wc[:64, 64:128] = Wr
        Wc[64, 64:128] = br
        return Wc

    Wl1 = W_l[:, Pm] * aab; bl1 = b_l[Pm] * aab
    Wr1 = W_r[:, Pm] * aab; br1 = b_r[Pm] * aab
    Wl2 = (W_l[Pm][:, Pm] * aab) / aab[:, None]
    Wr2 = (W_r[Pm][:, Pm] * aab) / aab[:, None]
    Wcat0 = wcat_of(Wl1, bl1, Wr1, br1)
    Wcat1 = wcat_of(Wl2, bl1, Wr2, br1)
    bias0 = np.tile((bias[Pm] * aab)[None, :], (128, 1)).astype(np.float32)
    bias1 = np.tile(bias[Pm][None, :], (128, 1)).astype(np.float32)
    rat = np.tile((1.0 / aab)[None, :], (128, 1)).astype(np.float32)
    ident = np.eye(128, dtype=np.float32)
    poison = np.zeros((1, 128), np.float16)
    poison[0, 64] = POISON_A

    in_maps = []
    for m in range(M):
        xT0 = np.zeros((65, PC), np.float16)
        xT0[0:64, 0:NS] = x[T['node_order'][m]].T
        xT0[64, :] = 1.0
        pcm = T['percore'][m]
        im = dict(xT0=xT0, Wcat0=Wcat0.astype(np.float16), Wcat1=Wcat1, bias0=bias0, bias1=bias1,
                  rat=rat, ident=ident, poison=poison,
                  vxidx=pcm['vxidx'], mB=pcm['mB'])
        for c in range(NW):
            im[f'sidx{c}'] = pcm['sidx'][c]
        in_maps.append(im)

    if os.environ.get('BASS_GAT_SIM'):
        from concourse import bass_interp
        sim = bass_interp.MultiCoreSim(nc, M)
        for m in range(M):
            for k, v in in_maps[m].items():
                sim.cores[m].tensor(k)[:] = v
        sim.simulate()
        results = [{'h2': sim.cores[m].mem_tensor('h2')} for m in range(M)]
    else:
        from concourse.bass_utils import run_bass_kernel_spmd
        trace = bool(os.environ.get('BASS_GAT_TRACE'))
        res = run_bass_kernel_spmd(nc, in_maps, list(range(M)), trace=trace)
        if trace:
            print(f"[hw] exec_time_ns: {res.exec_time_ns}")
            print(f"HW exec time: {res.exec_time_ns} ns")
        results = res.results

    h2 = np.zeros((N, D), np.float32)
    for m in range(M):
        h2[np.ix_(T['node_order'][m], Pm)] = results[m]['h2'][:NS]
    return x + h2


# revision 22
# speedup vs baseline: 1.0850x; 1.0151x over previous
"""GATv2 x2 + residual on 8 TRN2 NeuronCores (Bass/Tile).

Strategy (self-contained; N=100000, D=64, E=1700000):

- Nodes are assigned to the 8 cores by a greedy balancer (12500 each),
  then ordered per-core by descending in-degree ("device order"). All
  device tables (strips, nd accumulators, output) use this order, so
  per-destination loads are plain sequential DMAs instead of gathers.
- |att| is folded into the weights and features are permuted so att>0
  features come first. The per-edge score uses the exact identity
  lrelu_0.2(z) = 0.6 z + 0.4|z|:
      score = 0.6 (a_src + b_dst) + 0.4 (sum_pos|z| - sum_neg|z|)
  where a = sum_pos xl - sum_neg xl and b likewise for xr are per-node
  scalars computed once in the matmul phase and carried as column 64 of
  the fp16 strips. This removes the per-slot leaky-relu pass entirely;
  the |.| sums use tensor_reduce(apply_absolute_value=True).
- All per-edge tensors are fp16 (2x DVE throughput on packed ops); the
  weighted message sum uses the z-trick
      num = sum ex*z - den*xr            (z = xl + xr)
  so the gathered xl tile is only read once (by the z add).
- Tables are fp16 with 128-wide rows ([xl(64) | a | pad]): one 256B
  gather descriptor per edge, same as f32/64-wide. Gathers cycle over 4
  SWDGE queues with a 64KB descriptor ring.
- Self-loops never enter the edge streams; slot padding gathers a
  poison row (a = -1e4) whose score underflows exp to exactly 0.
- Per-row softmax over destination-major slots [128 dst x slots];
  heavy rows overflow to per-core compacted virtual rows (nd_v).
  Virtual groups run FIRST; each main group is immediately followed by
  its merge (consuming the nd accumulator tile directly from SBUF) and,
  in layer 0, by the layer-1 matmul chunk for the same columns, whose
  moving operand the merge assembles in SBUF from its PE transposes --
  no nd/hT DRAM round-trips and no serial merge/matmul phases. Merges
  of groups with no virtual rows on any core skip the nd_v gather.
- exp(score) is expanded across the 64 features by the Scalar engine so
  the DVE weight multiply is fully packed; slot sums use a halving tree
  of packed adds instead of strided reduces. Layer-0 matmuls run in
  fp16 (1 PE pass). Layer 2's merge writes the final features; the
  host adds the residual.
"""
import os
import numpy as np

N = 100000
D = 64
M = 8
NS = 12500              # real nodes per core
PC = 12544              # padded strip rows (98 * 128)
NW = 4                  # gather windows (core pairs)
WIN = 2 * PC            # rows per window
NEG = 0.2
CAPQ = 0.75             # slot cap quantile within a group
G = 4                   # 128-row blocks per group
RG = 128 * G            # rows per group (512)
NGM = 25                # main groups (25*512 = 12800 >= NS)
MROWS = NGM * RG        # 12800
TBL = M * PC            # table rows (100352)
POISON_A = -10000.0
PIDX = NS               # window-local poison row (even strip, row NS)


# ----------------------------------------------------------------------
# host preprocessing
# ----------------------------------------------------------------------

def _assign_cores(src, dst):
    order = np.argsort(src, kind='stable')
    d_sorted = dst[order]
    starts = np.searchsorted(src[order], np.arange(N + 1))
    core = np.full(N, -1, np.int32)
    quota = np.full(M, NS, np.int64)
    cnt = np.zeros((N, NW), np.float32)
    outdeg = starts[1:] - starts[:-1]
    proc = np.argsort(-outdeg, kind='stable')
    pair_edges = np.zeros(NW, np.float64)
    for v in proc:
        ds = d_sorted[starts[v]:starts[v + 1]]
        costs = cnt[ds].sum(axis=0) if len(ds) else np.zeros(NW, np.float32)
        costs = costs + 1e-7 * pair_edges
        best = None
        for p in np.argsort(costs, kind='stable'):
            if quota[2 * p] > 0 or quota[2 * p + 1] > 0:
                best = int(p)
                break
        c0, c1 = 2 * best, 2 * best + 1
        c = c0 if quota[c0] >= quota[c1] else c1
        core[v] = c
        quota[c] -= 1
        if len(ds):
            cnt[ds, best] += 1.0
            pair_edges[best] += len(ds)
    return core


def _wrap_idx(flat):
    """dma_gather index layout: [128, n/16] int16, 16-wrapped, 8x replicated."""
    n = len(flat)
    assert n % 16 == 0
    w = flat.reshape(n // 16, 16).T
    return np.ascontiguousarray(np.tile(w, (8, 1)), dtype=np.int16)


def preprocess(x, edge_index):
    src, dst = np.asarray(edge_index[0]), np.asarray(edge_index[1])
    nonself = src != dst
    src, dst = src[nonself].astype(np.int64), dst[nonself].astype(np.int64)
    core = _assign_cores(np.asarray(edge_index[0]), np.asarray(edge_index[1]))
    win_of = core // 2

    # provisional per-core rank (any order), then degree-sort -> device order
    node_order = np.zeros((M, NS), np.int64)
    for m in range(M):
        ids = np.where(core == m)[0]
        node_order[m] = ids
    # per-node non-self in-degree
    indeg = np.bincount(dst, minlength=N)
    for m in range(M):
        o = np.argsort(-indeg[node_order[m]], kind='stable')
        node_order[m] = node_order[m][o]
    rank = np.zeros(N, np.int64)
    for m in range(M):
        rank[node_order[m]] = np.arange(NS)
    twl = (core % 2).astype(np.int64) * PC + rank       # window-local table row

    # per-core edge lists grouped by (dst rank, window)
    svals, bounds = [], []
    for m in range(M):
        em = np.where(core[dst] == m)[0]
        es, ed = src[em], dst[em]
        key = rank[ed] * NW + win_of[es]
        # ascending src within each (rank, window) segment: gather columns
        # then cluster into narrow table ranges (better HBM locality)
        ko = np.lexsort((twl[es], key))
        svals.append(twl[es[ko]])
        bounds.append(np.searchsorted(key[ko], np.arange(NS * NW + 1)))

    cnts = np.zeros((M, NS, NW), np.int64)
    for m in range(M):
        b = bounds[m]
        cnts[m] = (b[1:] - b[:-1]).reshape(NS, NW)

    # shared main-group slot caps (device order = degree order)
    S_main = np.zeros((NGM, NW), np.int32)
    for g in range(NGM):
        caps = np.zeros((M, NW), np.int32)
        for m in range(M):
            rows = cnts[m][g * RG:(g + 1) * RG]
            if len(rows) == 0:
                continue
            caps[m] = np.ceil(np.quantile(rows, CAPQ, axis=0)).astype(np.int32)
        S_main[g] = caps.max(axis=0)
    S_main = np.maximum(S_main, 1)

    # overflow -> per-core compacted virtual rows
    gidx = np.minimum(np.arange(NS) // RG, NGM - 1)
    ov = np.maximum(cnts - S_main[gidx][None, :, :], 0)   # [M, NS, NW]
    virt = []
    for m in range(M):
        v = np.where(ov[m].sum(axis=1) > 0)[0]
        virt.append(v[np.argsort(-ov[m][v].sum(axis=1), kind='stable')])
    NV = max(len(v) for v in virt)
    NGV = (NV + RG - 1) // RG
    S_virt = np.zeros((NGV, NW), np.int32)
    for g in range(NGV):
        mx = np.zeros(NW, np.int32)
        for m in range(M):
            v = virt[m][g * RG:(g + 1) * RG]
            if len(v):
                mx = np.maximum(mx, ov[m][v].max(axis=0))
        S_virt[g] = np.maximum(mx, 1)

    NG = NGM + NGV
    S = np.concatenate([S_main, S_virt], axis=0)         # [NG, NW]
    R = NG * RG
    ZR = R                                               # zero-row id

    hv_glob = np.zeros(NGM, bool)
    percore = []
    for m in range(M):
        b = bounds[m]
        sv = svals[m]
        vr = virt[m]
        virtrow = np.full(NS, NGV * RG, np.int64)  # default: zero row (virt-local)
        sidx = [[] for _ in range(NW)]
        vxidx = []
        for g in range(NG):
            is_v = g >= NGM
            for c in range(NW):
                S_c = int(S[g, c])
                flat = np.full(128 * G * S_c, PIDX, np.int16)
                for gg in range(G):
                    for p in range(128):
                        i = gg * 128 + p
                        if not is_v:
                            r = g * RG + i
                            if r >= NS:
                                continue
                        else:
                            gi = (g - NGM) * RG + i
                            if gi >= len(vr):
                                continue
                            r = vr[gi]
                            if c == 0:
                                virtrow[r] = (g - NGM) * RG + i
                        lo, hi = b[r * NW + c], b[r * NW + c + 1]
                        if is_v:
                            lo = lo + int(S_main[min(r // RG, NGM - 1), c])
                        seg = sv[lo:min(hi, lo + S_c)]
                        for s_i, v_ in enumerate(seg):
                            flat[(gg * S_c + s_i) * 128 + p] = v_
                sidx[c].append(_wrap_idx(flat))
            if is_v:
                vx = np.zeros(RG, np.int16)
                gi0 = (g - NGM) * RG
                for i in range(RG):
                    gi = gi0 + i
                    vx[i] = vr[gi] if gi < len(vr) else NS  # pad: xr row NS
                vxidx.append(_wrap_idx(vx))
        mB = np.full(MROWS, NGV * RG, np.int16)
        mB[:NS] = virtrow
        vranks = np.where(virtrow != NGV * RG)[0]
        hv_glob[np.minimum(vranks // RG, NGM - 1)] = True
        percore.append(dict(
            sidx=[np.concatenate(s, axis=1) for s in sidx],
            vxidx=(np.concatenate(vxidx, axis=1) if NGV else
                   np.zeros((128, 32), np.int16)),
            mB=np.concatenate([_wrap_idx(mB[g * RG:(g + 1) * RG])
                               for g in range(NGM)], axis=1),
        ))
    return dict(core=core, node_order=node_order, S=S, NGV=NGV, NG=NG,
                R=R, percore=percore, hv=hv_glob.tolist())


# ----------------------------------------------------------------------
# device program
# ----------------------------------------------------------------------

def build_program(S, NGV, NPOS, HV=None):
    from concourse import bass, mybir, tile
    from concourse import bacc
    f32 = mybir.dt.float32
    f16 = mybir.dt.float16
    i16 = mybir.dt.int16
    NG = NGM + NGV
    R = NG * RG
    CC = [sum(8 * G * int(S[g][c]) for g in range(NG)) for c in range(NW)]
    assert 0 < NPOS < 64
    if HV is None:
        HV = [True] * NGM

    nc = bacc.Bacc(num_swdge_queues=4, dynamic_dma_scratch_size=64512)
    P = {}
    P['xT0'] = nc.declare_dram_parameter("xT0", [65, PC], f16, isOutput=False)
    P['Wcat0'] = nc.declare_dram_parameter("Wcat0", [65, 128], f16, isOutput=False)
    P['Wcat1'] = nc.declare_dram_parameter("Wcat1", [65, 128], f32, isOutput=False)
    P['bias0'] = nc.declare_dram_parameter("bias0", [128, 64], f32, isOutput=False)
    P['bias1'] = nc.declare_dram_parameter("bias1", [128, 64], f32, isOutput=False)
    P['rat'] = nc.declare_dram_parameter("rat", [128, 64], f32, isOutput=False)
    P['ident'] = nc.declare_dram_parameter("ident", [128, 128], f32, isOutput=False)
    P['poison'] = nc.declare_dram_parameter("poison", [1, 128], f16, isOutput=False)
    for c in range(NW):
        P[f'sidx{c}'] = nc.declare_dram_parameter(f"sidx{c}", [128, CC[c]], i16, isOutput=False)
    P['vxidx'] = nc.declare_dram_parameter("vxidx", [128, max(32 * NGV, 32)], i16, isOutput=False)
    P['mB'] = nc.declare_dram_parameter("mB", [128, 32 * NGM], i16, isOutput=False)
    h2out = nc.declare_dram_parameter("h2", [MROWS, 64], f32, isOutput=True)

    strip = [nc.dram_tensor(f"strip{l}", [MROWS, 128], f16) for l in range(2)]
    xr_t = [nc.dram_tensor(f"xr{l}", [MROWS, 128], f16) for l in range(2)]
    table = [nc.dram_tensor(f"table{l}", [TBL, 128], f16, addr_space="Shared")
             for l in range(2)]
    nd_v = [nc.dram_tensor(f"ndv{l}", [NGV * RG + 128, 128], f16) for l in range(2)]

    from contextlib import ExitStack
    _regs = {}

    def nireg(v):
        if v not in _regs:
            r = nc.gpsimd.alloc_register(f"ni{v}")
            nc.gpsimd.reg_mov(r, v)
            _regs[v] = r
        return _regs[v]

    _qctr = [0]

    def nextq():
        q = _qctr[0] % 4
        _qctr[0] += 1
        return q

    with tile.TileContext(nc) as tc, ExitStack() as es, \
            nc.allow_low_precision(reason="fp16 softmax accumulators"):
        cpool = es.enter_context(tc.tile_pool(name="const", bufs=1))
        wcat = [cpool.tile([65, 128], f16 if i == 0 else f32, name=f"wcat{i}")
                for i in range(2)]
        biasT = [cpool.tile([128, 64], f32, name=f"biasT{i}") for i in range(2)]
        rat = cpool.tile([128, 64], f32)
        ident = cpool.tile([128, 128], f32)
        poison = cpool.tile([1, 128], f16)
        for l in range(2):
            nc.sync.dma_start(out=wcat[l][:], in_=P[f'Wcat{l}'][:, :])
            nc.sync.dma_start(out=biasT[l][:], in_=P[f'bias{l}'][:, :])
        nc.sync.dma_start(out=rat[:], in_=P['rat'][:, :])
        nc.sync.dma_start(out=ident[:], in_=P['ident'][:, :])
        nc.sync.dma_start(out=poison[:], in_=P['poison'][:, :])
        # zero-rows of nd tables (fp16); ones-row of hT
        zt = cpool.tile([128, 128], f16)
        nc.vector.memset(zt[:], 0.0)
        zt2 = cpool.tile([128, 256], f16)
        nc.vector.memset(zt2[:], 0.0)
        for l in range(2):
            nc.sync.dma_start(out=nd_v[l][NGV * RG:NGV * RG + 128, :], in_=zt[:])
            # zero tail rows PC..MROWS (keep pad-rank self scores finite)
            for tn in (strip[l], xr_t[l]):
                nc.sync.dma_start(
                    out=tn[PC:PC + 256, :].rearrange("(t p) d -> p t d", p=128),
                    in_=zt2[:, :].rearrange("p (t d) -> p t d", d=128))

        mmpool = es.enter_context(tc.tile_pool(name="mm", bufs=2))
        pspool = es.enter_context(tc.tile_pool(name="ps", bufs=4, space="PSUM"))
        xlpool = es.enter_context(tc.tile_pool(name="xl", bufs=3))
        hpool = es.enter_context(tc.tile_pool(name="h", bufs=2))
        apool = es.enter_context(tc.tile_pool(name="acc", bufs=2))
        spool = es.enter_context(tc.tile_pool(name="small", bufs=2))
        mpool = es.enter_context(tc.tile_pool(name="merge", bufs=2))
        epool = es.enter_context(tc.tile_pool(name="exw", bufs=1))
        xpool = es.enter_context(tc.tile_pool(name="xr", bufs=3))

        mm_groups = [4] * (PC // 512) + ([(PC % 512) // 128] if PC % 512 else [])
        NL = int(os.environ.get('BASS_GAT_LAYERS', '2'))

        def mm_chunk(l, mg, xt_in=None):
            tw = mm_groups[mg]
            c0 = mg * 512
            if xt_in is None:
                assert l == 0
                xt = mmpool.tile([65, tw * 128], f16, tag="xt0", name="xt")
                nc.sync.dma_start(out=xt[:], in_=P['xT0'][0:65, c0:c0 + tw * 128])
            else:
                xt = xt_in
            sb = mmpool.tile([128, tw, 128], f32, tag="mmsb", name="sb")
            for t in range(tw):
                ps = pspool.tile([128, 128], f32, tag="mmps", name="ps")
                nc.tensor.matmul(ps[:], xt[:, t * 128:(t + 1) * 128],
                                 wcat[l][:], start=True, stop=True)
                nc.scalar.copy(sb[:, t, :], ps[:])
            # per-node score scalars a (from xl cols) and b (from xr cols)
            red = mmpool.tile([128, 4, tw], f32, tag="mmred", name="red")
            nc.vector.tensor_reduce(red[:, 0, :], sb[:, :, 0:NPOS],
                                    axis=mybir.AxisListType.X, op=mybir.AluOpType.add)
            nc.vector.tensor_reduce(red[:, 1, :], sb[:, :, NPOS:64],
                                    axis=mybir.AxisListType.X, op=mybir.AluOpType.add)
            nc.vector.tensor_reduce(red[:, 2, :], sb[:, :, 64:64 + NPOS],
                                    axis=mybir.AxisListType.X, op=mybir.AluOpType.add)
            nc.vector.tensor_reduce(red[:, 3, :], sb[:, :, 64 + NPOS:128],
                                    axis=mybir.AxisListType.X, op=mybir.AluOpType.add)
            stF = mmpool.tile([128, tw, 128], f16, tag="mmst", name="stF")
            xrF = mmpool.tile([128, tw, 128], f16, tag="mmxr", name="xrF")
            nc.scalar.copy(stF[:, :, 0:64], sb[:, :, 0:64])
            nc.scalar.copy(xrF[:, :, 0:64], sb[:, :, 64:128])
            nc.vector.tensor_sub(stF[:, :, 64], red[:, 0, :], red[:, 1, :])
            nc.vector.tensor_sub(xrF[:, :, 64], red[:, 2, :], red[:, 3, :])
            dst_xl = strip[l][c0:c0 + tw * 128, :].rearrange(
                "(t p) d -> p t d", p=128)
            dst_xr = xr_t[l][c0:c0 + tw * 128, :].rearrange(
                "(t p) d -> p t d", p=128)
            nc.scalar.dma_start(out=dst_xl, in_=stF[:])
            nc.sync.dma_start(out=dst_xr, in_=xrF[:])

        def merge_group(l, g, nd, hv):
            if hv:
                ib = mpool.tile([128, 32], i16, tag="ib", name="ib")
                nc.sync.dma_start(out=ib[:], in_=P['mB'][:, 32 * g:32 * (g + 1)])
                gb = mpool.tile([128, G, 128], f16, tag="gb", name="gb")
                nc.gpsimd.dma_gather(out_ap=gb[:], in_ap=nd_v[l][:, :], idxs_ap=ib[:],
                                     num_idxs=RG, num_idxs_reg=nireg(RG),
                                     elem_size=128, queue_num=nextq())
                sm = mpool.tile([128, G, 65], f32, tag="sm", name="sm")
                nc.vector.tensor_add(sm[:], nd[:], gb[:, :, 0:65])
            else:
                sm = mpool.tile([128, G, 65], f32, tag="sm", name="sm")
                nc.scalar.copy(sm[:], nd[:])
            rc = mpool.tile([128, G, 1], f32, tag="rc", name="rc")
            nc.vector.reciprocal(rc[:], sm[:, :, 64:65])
            hm = mpool.tile([128, G, 64], f32, tag="hm", name="hm")
            nc.vector.tensor_mul(hm[:], sm[:, :, 0:64],
                                 rc[:, :, :].to_broadcast([128, G, 64]))
            if l == 1:
                nc.vector.tensor_mul(hm[:], hm[:],
                                     rat[:, :].unsqueeze(1).to_broadcast([128, G, 64]))
            nc.vector.tensor_add(hm[:], hm[:],
                                 biasT[l][:, :].unsqueeze(1).to_broadcast([128, G, 64]))
            if l == 0:
                # write PE-transposed features straight into layer 1's moving
                # matmul operand (no hT round-trip through DRAM)
                tw = mm_groups[g] if g < len(mm_groups) else 0
                xt = mmpool.tile([65, 512], f32, tag="xt", name="xt")
                nc.vector.memset(xt[64:65, :], 1.0)
                for t in range(tw):
                    pst = pspool.tile([64, 128], f32, tag="pst", name="pst")
                    nc.tensor.transpose(pst[:], hm[:, t, :], ident[:])
                    nc.scalar.copy(xt[0:64, t * 128:(t + 1) * 128], pst[:])
                return xt
            else:
                dst_h = h2out[g * RG:(g + 1) * RG, :].rearrange(
                    "(t p) d -> p t d", p=128)
                nc.scalar.dma_start(out=dst_h, in_=hm[:])
                return None

        def slot_group(l, g, coff):
            is_v = g >= NGM
            nws = G * int(S[g].sum())
            den = apool.tile([128, G], f16, tag="den")
            nd = apool.tile([128, G, 65], f16, tag="nd")
            if not is_v:
                xr128 = xpool.tile([128, G, 65], f16, tag="xr")
                nc.sync.dma_start(
                    out=xr128[:],
                    in_=xr_t[l][g * RG:(g + 1) * RG, 0:65].rearrange(
                        "(t p) d -> p t d", p=128))
                xlo = xpool.tile([128, G, 65], f16, tag="xlo")
                nc.sync.dma_start(
                    out=xlo[:],
                    in_=strip[l][g * RG:(g + 1) * RG, 0:65].rearrange(
                        "(t p) d -> p t d", p=128))
                xr64 = xr128[:, :, 0:64]
                bcol = xr128[:, :, 64:65]
            else:
                vxi = spool.tile([128, 32], i16, tag="vxi")
                gv = g - NGM
                nc.sync.dma_start(out=vxi[:], in_=P['vxidx'][:, 32 * gv:32 * (gv + 1)])
                vxr = xpool.tile([128, G, 128], f16, tag="vxr")
                nc.gpsimd.dma_gather(out_ap=vxr[:], in_ap=xr_t[l][:, :],
                                     idxs_ap=vxi[:], num_idxs=RG,
                                     num_idxs_reg=nireg(RG), elem_size=128,
                                     queue_num=nextq())
                xr64 = vxr[:, :, 0:64]
                bcol = vxr[:, :, 64:65]
            # idx loads ride the Scalar HWDGE queue so they are never stuck
            # behind Sync-queue waits; gathers fill one whole-group tile
            xlg = xlpool.tile([128, nws, 128], f16, tag="xl")
            spans = []
            qo = 0
            for c in range(NW):
                S_c = int(S[g][c])
                w = 8 * G * S_c
                it = spool.tile([128, w], i16, tag=f"it{c}")
                nc.sync.dma_start(out=it[:], in_=P[f'sidx{c}'][:, coff[c]:coff[c] + w])
                for q0 in range(0, G * S_c, 8):
                    qn = min(8, G * S_c - q0)
                    nc.gpsimd.dma_gather(
                        out_ap=xlg[:, qo + q0:qo + q0 + qn, :],
                        in_ap=table[l][c * WIN:(c + 1) * WIN, :],
                        idxs_ap=it[:, q0 * 8:(q0 + qn) * 8],
                        num_idxs=128 * qn, num_idxs_reg=nireg(128 * qn),
                        elem_size=128, queue_num=nextq())
                spans.append((qo, S_c))
                qo += G * S_c

            z = hpool.tile([128, nws, 64], f16, tag="z")
            sc = spool.tile([128, nws], f16, tag="sc")
            scn = spool.tile([128, nws], f16, tag="scn")
            ab = spool.tile([128, nws], f16, tag="ab")
            for (qo, S_c) in spans:
                z4 = z[:, qo:qo + G * S_c, :].rearrange("p (g s) d -> p g s d", g=G)
                xlg4 = xlg[:, qo:qo + G * S_c, 0:64].rearrange(
                    "p (g s) d -> p g s d", g=G)
                xrb = xr64.unsqueeze(2).to_broadcast([128, G, S_c, 64])
                nc.vector.tensor_add(z4, xlg4, xrb)
                sc3 = sc[:, qo:qo + G * S_c].rearrange("p (g s) -> p g s", g=G)
                scn3 = scn[:, qo:qo + G * S_c].rearrange("p (g s) -> p g s", g=G)
                nc.vector.tensor_reduce(sc3, z4[:, :, :, 0:NPOS],
                                        axis=mybir.AxisListType.X,
                                        op=mybir.AluOpType.add,
                                        apply_absolute_value=True)
                nc.vector.tensor_reduce(scn3, z4[:, :, :, NPOS:64],
                                        axis=mybir.AxisListType.X,
                                        op=mybir.AluOpType.add,
                                        apply_absolute_value=True)
                ab3 = ab[:, qo:qo + G * S_c].rearrange("p (g s) -> p g s", g=G)
                a3 = xlg[:, qo:qo + G * S_c, 64].rearrange("p (g s) -> p g s", g=G)
                nc.vector.tensor_add(ab3, a3,
                                     bcol[:, :, 0].unsqueeze(2).to_broadcast(
                                         [128, G, S_c]))
            nc.vector.tensor_sub(sc[:], sc[:], scn[:])
            # score = 0.4 * (sc + 1.5 * ab); exp via ACT scale
            nc.vector.scalar_tensor_tensor(
                sc[:], ab[:], 1.5, sc[:],
                mybir.AluOpType.mult, mybir.AluOpType.add)
            ex = spool.tile([128, nws], f16, tag="ex")
            nc.scalar.activation(ex[:], sc[:], mybir.ActivationFunctionType.Exp,
                                 scale=0.4)
            dtmp4 = spool.tile([128, NW, G], f16, tag="dtmp4")
            ntmp4 = spool.tile([128, NW, G, 64], f16, tag="ntmp4")
            for ci, (qo, S_c) in enumerate(spans):
                ex3 = ex[:, qo:qo + G * S_c].rearrange("p (g s) -> p g s", g=G)
                nc.vector.tensor_reduce(dtmp4[:, ci], ex3,
                                        axis=mybir.AxisListType.X,
                                        op=mybir.AluOpType.add)
                z4 = z[:, qo:qo + G * S_c, :].rearrange("p (g s) d -> p g s d", g=G)
                # expand ex across features on the Scalar engine so the DVE
                # multiply is fully packed, then sum slots by a halving tree
                # of packed adds (both ~2x faster than strided reduce)
                exw = epool.tile([128, G, S_c, 64], f16, tag="exw")
                nc.scalar.copy(exw[:], ex3.unsqueeze(3).to_broadcast(
                    [128, G, S_c, 64]))
                nc.vector.tensor_mul(z4, z4, exw[:])
                m = S_c
                while m > 2:
                    h = m // 2
                    nc.vector.tensor_add(z4[:, :, 0:h, :], z4[:, :, 0:h, :],
                                         z4[:, :, m - h:m, :])
                    m = m - h
                nt = ntmp4[:, ci].unsqueeze(2)
                if m == 2:
                    nc.vector.tensor_add(nt, z4[:, :, 0:1, :], z4[:, :, 1:2, :])
                else:
                    nc.vector.tensor_copy(nt, z4[:, :, 0:1, :])
            dtmp = spool.tile([128, G], f16, tag="dtmp")
            nc.vector.tensor_add(ntmp4[:, 0], ntmp4[:, 0], ntmp4[:, 1])
            nc.vector.tensor_add(ntmp4[:, 2], ntmp4[:, 2], ntmp4[:, 3])
            nc.vector.tensor_add(nd[:, :, 0:64], ntmp4[:, 0], ntmp4[:, 2])
            nc.vector.tensor_add(dtmp4[:, 0], dtmp4[:, 0], dtmp4[:, 1])
            nc.vector.tensor_add(dtmp4[:, 2], dtmp4[:, 2], dtmp4[:, 3])
            nc.vector.tensor_add(dtmp[:], dtmp4[:, 0], dtmp4[:, 2])
            if not is_v:
                # self-loop: z_self = xlo + xr, score from a_own + b
                zs = spool.tile([128, G, 64], f16, tag="zs")
                nc.vector.tensor_add(zs[:], xlo[:, :, 0:64], xr64)
                scs = spool.tile([128, 2, G], f16, tag="scs")
                nc.vector.tensor_reduce(scs[:, 0], zs[:, :, 0:NPOS],
                                        axis=mybir.AxisListType.X,
                                        op=mybir.AluOpType.add,
                                        apply_absolute_value=True)
                nc.vector.tensor_reduce(scs[:, 1], zs[:, :, NPOS:64],
                                        axis=mybir.AxisListType.X,
                                        op=mybir.AluOpType.add,
                                        apply_absolute_value=True)
                nc.vector.tensor_sub(scs[:, 0], scs[:, 0], scs[:, 1])
                abs_ = spool.tile([128, G], f16, tag="abs")
                nc.vector.tensor_add(abs_[:], xlo[:, :, 64], bcol[:, :, 0])
                nc.vector.scalar_tensor_tensor(
                    scs[:, 0], abs_[:], 1.5, scs[:, 0],
                    mybir.AluOpType.mult, mybir.AluOpType.add)
                exs = spool.tile([128, G], f16, tag="exs")
                nc.scalar.activation(exs[:], scs[:, 0],
                                     mybir.ActivationFunctionType.Exp, scale=0.4)
                nc.vector.tensor_add(den[:], exs[:], dtmp[:])
                # num += exs * z_self
                nc.vector.tensor_mul(zs[:], zs[:],
                                     exs[:, :].unsqueeze(2).to_broadcast([128, G, 64]))
                nc.vector.tensor_add(nd[:, :, 0:64], nd[:, :, 0:64], zs[:])
            else:
                nc.scalar.copy(den[:], dtmp[:])
            # z-trick correction: num -= den * xr
            corr = spool.tile([128, G, 64], f16, tag="corr")
            nc.vector.tensor_mul(corr[:], xr64,
                                 den[:, :].unsqueeze(2).to_broadcast([128, G, 64]))
            nc.vector.tensor_sub(nd[:, :, 0:64], nd[:, :, 0:64], corr[:])
            nc.scalar.copy(nd[:, :, 64:65], den[:, :].unsqueeze(2))
            if is_v:
                gv = g - NGM
                dst_nd = nd_v[l][gv * RG:(gv + 1) * RG, 0:65].rearrange(
                    "(t p) d -> p t d", p=128)
                nc.scalar.dma_start(out=dst_nd, in_=nd[:])
            return nd

        for l in range(NL):
            # ---- matmul phase (layer 0 only; layer 1 fused into merges) ----
            if l == 0:
                for mg in range(len(mm_groups)):
                    mm_chunk(l, mg)
            # poison row for padded slots (window-local row NS of even strip)
            nc.sync.dma_start(out=strip[l][NS:NS + 1, :], in_=poison[:])
            # ---- all-gather the table ----
            nc.gpsimd.collective_compute(
                "AllGather", mybir.AluOpType.bypass,
                replica_groups=[list(range(M))],
                ins=[strip[l][0:PC, :]], outs=[table[l][:, :]])

            # ---- slot offsets ----
            coff = [0] * NW
            coff_at = {}
            for g in range(NG):
                coff_at[g] = list(coff)
                for c in range(NW):
                    coff[c] += 8 * G * int(S[g][c])
            # ---- virtual groups first, then main + fused merge (+ next mm) --
            for g in range(NGM, NG):
                slot_group(l, g, coff_at[g])
            for g in range(NGM):
                nd = slot_group(l, g, coff_at[g])
                xt = merge_group(l, g, nd, HV[g])
                if l == 0 and NL > 1:
                    mm_chunk(1, g, xt_in=xt)

    nc.compile()
    return nc


# ----------------------------------------------------------------------
# entry point
# ----------------------------------------------------------------------

def kernel(**inputs):
    x = np.asarray(inputs['x'], np.float32)
    ei = np.asarray(inputs['edge_index'])
    W_l = np.asarray(inputs['W_l'], np.float64)
    b_l = np.asarray(inputs['b_l'], np.float64)
    W_r = np.asarray(inputs['W_r'], np.float64)
    b_r = np.asarray(inputs['b_r'], np.float64)
    att = np.asarray(inputs['att'], np.float64)
    bias = np.asarray(inputs['bias'], np.float64)

    T = preprocess(x, ei)

    # fold |att| into the weights; permute features so att>0 comes first
    Pm = np.concatenate([np.where(att > 0)[0], np.where(att <= 0)[0]])
    NPOS = int((att > 0).sum())
    aab = np.abs(att[Pm])
    aab[aab == 0] = 1.0

    nc = build_program(T['S'], T['NGV'], NPOS, T['hv'])

    def wcat_of(Wl, bl, Wr, br):
        Wc = np.zeros((65, 128), np.float32)
        Wc[:64, 0:64] = Wl
        Wc[64, 0:64] = bl
        Wc[:64, 64:128] = Wr
        Wc[64, 64:128] = br
        return Wc

    Wl1 = W_l[:, Pm] * aab; bl1 = b_l[Pm] * aab
    Wr1 = W_r[:, Pm] * aab; br1 = b_r[Pm] * aab
    Wl2 = (W_l[Pm][:, Pm] * aab) / aab[:, None]
    Wr2 = (W_r[Pm][:, Pm] * aab) / aab[:, None]
    Wcat0 = wcat_of(Wl1, bl1, Wr1, br1)
    Wcat1 = wcat_of(Wl2, bl1, Wr2, br1)
    bias0 = np.tile((bias[Pm] * aab)[None, :], (128, 1)).astype(np.float32)
    bias1 = np.tile(bias[Pm][None, :], (128, 1)).astype(np.float32)
    rat = np.tile((1.0 / aab)[None, :], (128, 1)).astype(np.float32)
    ident = np.eye(128, dtype=np.float32)
    poison = np.zeros((1, 128), np.float16)
    poison[0, 64] = POISON_A

    in_maps = []
    for m in range(M):
        xT0 = np.zeros((65, PC), np.float16)
        xT0[0:64, 0:NS] = x[T['node_order'][m]].T
        xT0[64, :] = 1.0
        pcm = T['percore'][m]
        im = dict(xT0=xT0, Wcat0=Wcat0.astype(np.float16), Wcat1=Wcat1, bias0=bias0, bias1=bias1,
                  rat=rat, ident=ident, poison=poison,
                  vxidx=pcm['vxidx'], mB=pcm['mB'])
        for c in range(NW):
            im[f'sidx{c}'] = pcm['sidx'][c]
        in_maps.append(im)

    if os.environ.get('BASS_GAT_SIM'):
        from concourse import bass_interp
        sim = bass_interp.MultiCoreSim(nc, M)
        for m in range(M):
            for k, v in in_maps[m].items():
                sim.cores[m].tensor(k)[:] = v
        sim.simulate()
        results = [{'h2': sim.cores[m].mem_tensor('h2')} for m in range(M)]
    else:
        from concourse.bass_utils import run_bass_kernel_spmd
        trace = bool(os.environ.get('BASS_GAT_TRACE'))
        res = run_bass_kernel_spmd(nc, in_maps, list(range(M)), trace=trace)
        if trace:
            print(f"[hw] exec_time_ns: {res.exec_time_ns}")
            print(f"HW exec time: {res.exec_time_ns} ns")
        results = res.results

    h2 = np.zeros((N, D), np.float32)
    for m in range(M):
        h2[np.ix_(T['node_order'][m], Pm)] = results[m]['h2'][:NS]
    return x + h2


# revision 24
# speedup vs baseline: 1.1226x; 1.0347x over previous
"""GATv2 x2 + residual on 8 TRN2 NeuronCores (Bass/Tile).

Strategy (self-contained; N=100000, D=64, E=1700000):

- Nodes are assigned to the 8 cores by a greedy balancer (12500 each),
  then ordered per-core by descending in-degree ("device order"). All
  device tables (strips, nd accumulators, output) use this order, so
  per-destination loads are plain sequential DMAs instead of gathers.
- |att| is folded into the weights and features are permuted so att>0
  features come first. The per-edge score uses the exact identity
  lrelu_0.2(z) = 0.6 z + 0.4|z|:
      score = 0.6 (a_src + b_dst) + 0.4 (sum_pos|z| - sum_neg|z|)
  where a = sum_pos xl - sum_neg xl and b likewise for xr are per-node
  scalars computed once in the matmul phase and carried as column 64 of
  the fp16 strips. This removes the per-slot leaky-relu pass entirely;
  the |.| sums use tensor_reduce(apply_absolute_value=True).
- All per-edge tensors are fp16 (2x DVE throughput on packed ops); the
  weighted message sum uses the z-trick
      num = sum ex*z - den*xr            (z = xl + xr)
  so the gathered xl tile is only read once (by the z add).
- Tables are fp16 with 128-wide rows ([xl(64) | a | pad]): one 256B
  gather descriptor per edge, same as f32/64-wide. Gathers cycle over 4
  SWDGE queues with a 64KB descriptor ring.
- Self-loops never enter the edge streams; slot padding gathers a
  poison row (a = -1e4) whose score underflows exp to exactly 0.
- Per-row softmax over destination-major slots [128 dst x slots];
  heavy rows overflow to per-core compacted virtual rows (nd_v).
  Virtual groups run FIRST; each main group is immediately followed by
  its merge (consuming the nd accumulator tile directly from SBUF) and,
  in layer 0, by the layer-1 matmul chunk for the same columns, whose
  moving operand the merge assembles in SBUF from its PE transposes --
  no nd/hT DRAM round-trips and no serial merge/matmul phases. Merges
  of groups with no virtual rows on any core skip the nd_v gather.
- exp(score) is expanded across the 64 features by the Scalar engine so
  the DVE weight multiply is fully packed; slot sums use a halving tree
  of packed adds instead of strided reduces. Layer-0 matmuls run in
  fp16 (1 PE pass). Layer 2's merge writes the final features; the
  host adds the residual.
"""
import os
import numpy as np

N = 100000
D = 64
M = 8
NS = 12500              # real nodes per core
PC = 12544              # padded strip rows (98 * 128)
NW = 4                  # gather windows (core pairs)
WIN = 2 * PC            # rows per window
NEG = 0.2
CAPQ = 0.75             # slot cap quantile within a group
G = 4                   # 128-row blocks per group
RG = 128 * G            # rows per group (512)
NGM = 25                # main groups (25*512 = 12800 >= NS)
MROWS = NGM * RG        # 12800
TBL = M * PC            # table rows (100352)
POISON_A = -10000.0
PIDX = NS               # window-local poison row (even strip, row NS)


# ----------------------------------------------------------------------
# host preprocessing
# ----------------------------------------------------------------------

def _assign_cores(src, dst):
    order = np.argsort(src, kind='stable')
    d_sorted = dst[order]
    starts = np.searchsorted(src[order], np.arange(N + 1))
    core = np.full(N, -1, np.int32)
    quota = np.full(M, NS, np.int64)
    cnt = np.zeros((N, NW), np.float32)
    outdeg = starts[1:] - starts[:-1]
    proc = np.argsort(-outdeg, kind='stable')
    pair_edges = np.zeros(NW, np.float64)
    for v in proc:
        ds = d_sorted[starts[v]:starts[v + 1]]
        costs = cnt[ds].sum(axis=0) if len(ds) else np.zeros(NW, np.float32)
        costs = costs + 1e-7 * pair_edges
        best = None
        for p in np.argsort(costs, kind='stable'):
            if quota[2 * p] > 0 or quota[2 * p + 1] > 0:
                best = int(p)
                break
        c0, c1 = 2 * best, 2 * best + 1
        c = c0 if quota[c0] >= quota[c1] else c1
        core[v] = c
        quota[c] -= 1
        if len(ds):
            cnt[ds, best] += 1.0
            pair_edges[best] += len(ds)
    return core


def _wrap_idx(flat):
    """dma_gather index layout: [128, n/16] int16, 16-wrapped, 8x replicated."""
    n = len(flat)
    assert n % 16 == 0
    w = flat.reshape(n // 16, 16).T
    return np.ascontiguousarray(np.tile(w, (8, 1)), dtype=np.int16)


def preprocess(x, edge_index):
    src, dst = np.asarray(edge_index[0]), np.asarray(edge_index[1])
    nonself = src != dst
    src, dst = src[nonself].astype(np.int64), dst[nonself].astype(np.int64)
    core = _assign_cores(np.asarray(edge_index[0]), np.asarray(edge_index[1]))
    win_of = core // 2

    # provisional per-core rank (any order), then degree-sort -> device order
    node_order = np.zeros((M, NS), np.int64)
    for m in range(M):
        ids = np.where(core == m)[0]
        node_order[m] = ids
    # per-node non-self in-degree
    indeg = np.bincount(dst, minlength=N)
    for m in range(M):
        o = np.argsort(-indeg[node_order[m]], kind='stable')
        node_order[m] = node_order[m][o]
    rank = np.zeros(N, np.int64)
    for m in range(M):
        rank[node_order[m]] = np.arange(NS)
    twl = (core % 2).astype(np.int64) * PC + rank       # window-local table row

    # per-core edge lists grouped by (dst rank, window)
    svals, bounds = [], []
    for m in range(M):
        em = np.where(core[dst] == m)[0]
        es, ed = src[em], dst[em]
        key = rank[ed] * NW + win_of[es]
        # ascending src within each (rank, window) segment: gather columns
        # then cluster into narrow table ranges (better HBM locality)
        ko = np.lexsort((twl[es], key))
        svals.append(twl[es[ko]])
        bounds.append(np.searchsorted(key[ko], np.arange(NS * NW + 1)))

    cnts = np.zeros((M, NS, NW), np.int64)
    for m in range(M):
        b = bounds[m]
        cnts[m] = (b[1:] - b[:-1]).reshape(NS, NW)

    # shared main-group slot caps (device order = degree order)
    S_main = np.zeros((NGM, NW), np.int32)
    for g in range(NGM):
        caps = np.zeros((M, NW), np.int32)
        for m in range(M):
            rows = cnts[m][g * RG:(g + 1) * RG]
            if len(rows) == 0:
                continue
            caps[m] = np.ceil(np.quantile(rows, CAPQ, axis=0)).astype(np.int32)
        S_main[g] = caps.max(axis=0)
    S_main = np.maximum(S_main, 1)

    # overflow -> per-core compacted virtual rows
    gidx = np.minimum(np.arange(NS) // RG, NGM - 1)
    ov = np.maximum(cnts - S_main[gidx][None, :, :], 0)   # [M, NS, NW]
    virt = []
    for m in range(M):
        v = np.where(ov[m].sum(axis=1) > 0)[0]
        virt.append(v[np.argsort(-ov[m][v].sum(axis=1), kind='stable')])
    NV = max(len(v) for v in virt)
    NGV = (NV + RG - 1) // RG
    S_virt = np.zeros((NGV, NW), np.int32)
    for g in range(NGV):
        mx = np.zeros(NW, np.int32)
        for m in range(M):
            v = virt[m][g * RG:(g + 1) * RG]
            if len(v):
                mx = np.maximum(mx, ov[m][v].max(axis=0))
        S_virt[g] = np.maximum(mx, 1)

    NG = NGM + NGV
    S = np.concatenate([S_main, S_virt], axis=0)         # [NG, NW]
    R = NG * RG
    ZR = R                                               # zero-row id

    hv_glob = np.zeros(NGM, bool)
    percore = []
    for m in range(M):
        b = bounds[m]
        sv = svals[m]
        vr = virt[m]
        virtrow = np.full(NS, NGV * RG, np.int64)  # default: zero row (virt-local)
        sidx = [[] for _ in range(NW)]
        vxidx = []
        for g in range(NG):
            is_v = g >= NGM
            for c in range(NW):
                S_c = int(S[g, c])
                flat = np.full(128 * G * S_c, PIDX, np.int16)
                for gg in range(G):
                    for p in range(128):
                        i = gg * 128 + p
                        if not is_v:
                            r = g * RG + i
                            if r >= NS:
                                continue
                        else:
                            gi = (g - NGM) * RG + i
                            if gi >= len(vr):
                                continue
                            r = vr[gi]
                            if c == 0:
                                virtrow[r] = (g - NGM) * RG + i
                        lo, hi = b[r * NW + c], b[r * NW + c + 1]
                        if is_v:
                            lo = lo + int(S_main[min(r // RG, NGM - 1), c])
                        seg = sv[lo:min(hi, lo + S_c)]
                        for s_i, v_ in enumerate(seg):
                            flat[(gg * S_c + s_i) * 128 + p] = v_
                sidx[c].append(_wrap_idx(flat))
            if is_v:
                vx = np.zeros(RG, np.int16)
                gi0 = (g - NGM) * RG
                for i in range(RG):
                    gi = gi0 + i
                    vx[i] = vr[gi] if gi < len(vr) else NS  # pad: xr row NS
                vxidx.append(_wrap_idx(vx))
        mB = np.full(MROWS, NGV * RG, np.int16)
        mB[:NS] = virtrow
        vranks = np.where(virtrow != NGV * RG)[0]
        hv_glob[np.minimum(vranks // RG, NGM - 1)] = True
        percore.append(dict(
            sidx=[np.concatenate(s, axis=1) for s in sidx],
            vxidx=(np.concatenate(vxidx, axis=1) if NGV else
                   np.zeros((128, 32), np.int16)),
            mB=np.concatenate([_wrap_idx(mB[g * RG:(g + 1) * RG])
                               for g in range(NGM)], axis=1),
        ))
    return dict(core=core, node_order=node_order, S=S, NGV=NGV, NG=NG,
                R=R, percore=percore, hv=hv_glob.tolist())


# ----------------------------------------------------------------------
# device program
# ----------------------------------------------------------------------

def build_program(S, NGV, NPOS, HV=None):
    from concourse import bass, mybir, tile
    from concourse import bacc
    f32 = mybir.dt.float32
    f16 = mybir.dt.float16
    i16 = mybir.dt.int16
    NG = NGM + NGV
    R = NG * RG
    CC = [sum(8 * G * int(S[g][c]) for g in range(NG)) for c in range(NW)]
    assert 0 < NPOS < 64
    if HV is None:
        HV = [True] * NGM

    nc = bacc.Bacc(num_swdge_queues=4, dynamic_dma_scratch_size=64512)
    P = {}
    P['xT0'] = nc.declare_dram_parameter("xT0", [65, PC], f16, isOutput=False)
    P['Wcat0'] = nc.declare_dram_parameter("Wcat0", [65, 128], f16, isOutput=False)
    P['Wcat1'] = nc.declare_dram_parameter("Wcat1", [65, 128], f32, isOutput=False)
    P['bias0'] = nc.declare_dram_parameter("bias0", [128, 64], f32, isOutput=False)
    P['bias1'] = nc.declare_dram_parameter("bias1", [128, 64], f32, isOutput=False)
    P['rat'] = nc.declare_dram_parameter("rat", [128, 64], f32, isOutput=False)
    P['ident'] = nc.declare_dram_parameter("ident", [128, 128], f32, isOutput=False)
    P['poison'] = nc.declare_dram_parameter("poison", [1, 128], f16, isOutput=False)
    for c in range(NW):
        P[f'sidx{c}'] = nc.declare_dram_parameter(f"sidx{c}", [128, CC[c]], i16, isOutput=False)
    P['vxidx'] = nc.declare_dram_parameter("vxidx", [128, max(32 * NGV, 32)], i16, isOutput=False)
    P['mB'] = nc.declare_dram_parameter("mB", [128, 32 * NGM], i16, isOutput=False)
    h2out = nc.declare_dram_parameter("h2", [MROWS, 64], f32, isOutput=True)

    strip = [nc.dram_tensor(f"strip{l}", [MROWS, 128], f16) for l in range(2)]
    xr_t = [nc.dram_tensor(f"xr{l}", [MROWS, 128], f16) for l in range(2)]
    table = [nc.dram_tensor(f"table{l}", [TBL, 128], f16, addr_space="Shared")
             for l in range(2)]
    nd_v = [nc.dram_tensor(f"ndv{l}", [NGV * RG + 128, 128], f16) for l in range(2)]

    from contextlib import ExitStack
    _regs = {}

    def nireg(v):
        if v not in _regs:
            r = nc.gpsimd.alloc_register(f"ni{v}")
            nc.gpsimd.reg_mov(r, v)
            _regs[v] = r
        return _regs[v]

    _qctr = [0]

    def nextq():
        q = _qctr[0] % 4
        _qctr[0] += 1
        return q

    with tile.TileContext(nc) as tc, ExitStack() as es, \
            nc.allow_low_precision(reason="fp16 softmax accumulators"):
        cpool = es.enter_context(tc.tile_pool(name="const", bufs=1))
        wcat = [cpool.tile([65, 128], f16 if i == 0 else f32, name=f"wcat{i}")
                for i in range(2)]
        biasT = [cpool.tile([128, 64], f32, name=f"biasT{i}") for i in range(2)]
        rat = cpool.tile([128, 64], f32)
        ident = cpool.tile([128, 128], f32)
        poison = cpool.tile([1, 128], f16)
        for l in range(2):
            nc.sync.dma_start(out=wcat[l][:], in_=P[f'Wcat{l}'][:, :])
            nc.sync.dma_start(out=biasT[l][:], in_=P[f'bias{l}'][:, :])
        nc.sync.dma_start(out=rat[:], in_=P['rat'][:, :])
        nc.sync.dma_start(out=ident[:], in_=P['ident'][:, :])
        nc.sync.dma_start(out=poison[:], in_=P['poison'][:, :])
        # zero-rows of nd tables (fp16); ones-row of hT
        zt = cpool.tile([128, 128], f16)
        nc.vector.memset(zt[:], 0.0)
        zt2 = cpool.tile([128, 256], f16)
        nc.vector.memset(zt2[:], 0.0)
        for l in range(2):
            nc.sync.dma_start(out=nd_v[l][NGV * RG:NGV * RG + 128, :], in_=zt[:])
            # zero tail rows PC..MROWS (keep pad-rank self scores finite)
            for tn in (strip[l], xr_t[l]):
                nc.sync.dma_start(
                    out=tn[PC:PC + 256, :].rearrange("(t p) d -> p t d", p=128),
                    in_=zt2[:, :].rearrange("p (t d) -> p t d", d=128))

        mmpool = es.enter_context(tc.tile_pool(name="mm", bufs=2))
        pspool = es.enter_context(tc.tile_pool(name="ps", bufs=4, space="PSUM"))
        xlpool = es.enter_context(tc.tile_pool(name="xl", bufs=3))
        hpool = es.enter_context(tc.tile_pool(name="h", bufs=2))
        apool = es.enter_context(tc.tile_pool(name="acc", bufs=4))
        spool = es.enter_context(tc.tile_pool(name="small", bufs=2))
        mpool = es.enter_context(tc.tile_pool(name="merge", bufs=2))
        epool = es.enter_context(tc.tile_pool(name="exw", bufs=1))
        xpool = es.enter_context(tc.tile_pool(name="xr", bufs=3))

        mm_groups = [4] * (PC // 512) + ([(PC % 512) // 128] if PC % 512 else [])
        NL = int(os.environ.get('BASS_GAT_LAYERS', '2'))

        def mm_chunk(l, mg, xt_in=None):
            tw = mm_groups[mg]
            c0 = mg * 512
            if xt_in is None:
                assert l == 0
                xt = mmpool.tile([65, tw * 128], f16, tag="xt0", name="xt")
                nc.sync.dma_start(out=xt[:], in_=P['xT0'][0:65, c0:c0 + tw * 128])
            else:
                xt = xt_in
            sb = mmpool.tile([128, tw, 128], f32, tag="mmsb", name="sb")
            for t in range(tw):
                ps = pspool.tile([128, 128], f32, tag="mmps", name="ps")
                nc.tensor.matmul(ps[:], xt[:, t * 128:(t + 1) * 128],
                                 wcat[l][:], start=True, stop=True)
                nc.scalar.copy(sb[:, t, :], ps[:])
            # per-node score scalars a (from xl cols) and b (from xr cols)
            red = mmpool.tile([128, 4, tw], f32, tag="mmred", name="red")
            nc.vector.tensor_reduce(red[:, 0, :], sb[:, :, 0:NPOS],
                                    axis=mybir.AxisListType.X, op=mybir.AluOpType.add)
            nc.vector.tensor_reduce(red[:, 1, :], sb[:, :, NPOS:64],
                                    axis=mybir.AxisListType.X, op=mybir.AluOpType.add)
            nc.vector.tensor_reduce(red[:, 2, :], sb[:, :, 64:64 + NPOS],
                                    axis=mybir.AxisListType.X, op=mybir.AluOpType.add)
            nc.vector.tensor_reduce(red[:, 3, :], sb[:, :, 64 + NPOS:128],
                                    axis=mybir.AxisListType.X, op=mybir.AluOpType.add)
            stF = mmpool.tile([128, tw, 128], f16, tag="mmst", name="stF")
            xrF = mmpool.tile([128, tw, 128], f16, tag="mmxr", name="xrF")
            nc.scalar.copy(stF[:, :, 0:64], sb[:, :, 0:64])
            nc.scalar.copy(xrF[:, :, 0:64], sb[:, :, 64:128])
            nc.vector.tensor_sub(stF[:, :, 64], red[:, 0, :], red[:, 1, :])
            nc.vector.tensor_sub(xrF[:, :, 64], red[:, 2, :], red[:, 3, :])
            dst_xl = strip[l][c0:c0 + tw * 128, :].rearrange(
                "(t p) d -> p t d", p=128)
            dst_xr = xr_t[l][c0:c0 + tw * 128, :].rearrange(
                "(t p) d -> p t d", p=128)
            nc.scalar.dma_start(out=dst_xl, in_=stF[:])
            nc.sync.dma_start(out=dst_xr, in_=xrF[:])

        def merge_group(l, g, nd, hv):
            if hv:
                ib = mpool.tile([128, 32], i16, tag="ib", name="ib")
                nc.sync.dma_start(out=ib[:], in_=P['mB'][:, 32 * g:32 * (g + 1)])
                gb = mpool.tile([128, G, 128], f16, tag="gb", name="gb")
                nc.gpsimd.dma_gather(out_ap=gb[:], in_ap=nd_v[l][:, :], idxs_ap=ib[:],
                                     num_idxs=RG, num_idxs_reg=nireg(RG),
                                     elem_size=128, queue_num=nextq())
                sm = mpool.tile([128, G, 65], f32, tag="sm", name="sm")
                nc.vector.tensor_add(sm[:], nd[:], gb[:, :, 0:65])
            else:
                sm = mpool.tile([128, G, 65], f32, tag="sm", name="sm")
                nc.scalar.copy(sm[:], nd[:])
            rc = mpool.tile([128, G, 1], f32, tag="rc", name="rc")
            nc.vector.reciprocal(rc[:], sm[:, :, 64:65])
            hm = mpool.tile([128, G, 64], f32, tag="hm", name="hm")
            nc.vector.tensor_mul(hm[:], sm[:, :, 0:64],
                                 rc[:, :, :].to_broadcast([128, G, 64]))
            if l == 1:
                nc.vector.tensor_mul(hm[:], hm[:],
                                     rat[:, :].unsqueeze(1).to_broadcast([128, G, 64]))
            nc.vector.tensor_add(hm[:], hm[:],
                                 biasT[l][:, :].unsqueeze(1).to_broadcast([128, G, 64]))
            if l == 0:
                # write PE-transposed features straight into layer 1's moving
                # matmul operand (no hT round-trip through DRAM)
                tw = mm_groups[g] if g < len(mm_groups) else 0
                xt = mmpool.tile([65, 512], f32, tag="xt", name="xt")
                nc.vector.memset(xt[64:65, :], 1.0)
                for t in range(tw):
                    pst = pspool.tile([64, 128], f32, tag="pst", name="pst")
                    nc.tensor.transpose(pst[:], hm[:, t, :], ident[:])
                    nc.scalar.copy(xt[0:64, t * 128:(t + 1) * 128], pst[:])
                return xt
            else:
                dst_h = h2out[g * RG:(g + 1) * RG, :].rearrange(
                    "(t p) d -> p t d", p=128)
                nc.scalar.dma_start(out=dst_h, in_=hm[:])
                return None

        def slot_group(l, g, coff):
            is_v = g >= NGM
            nws = G * int(S[g].sum())
            den = apool.tile([128, G], f16, tag="den")
            nd = apool.tile([128, G, 65], f16, tag="nd")
            if not is_v:
                xr128 = xpool.tile([128, G, 65], f16, tag="xr")
                nc.sync.dma_start(
                    out=xr128[:],
                    in_=xr_t[l][g * RG:(g + 1) * RG, 0:65].rearrange(
                        "(t p) d -> p t d", p=128))
                xlo = xpool.tile([128, G, 65], f16, tag="xlo")
                nc.sync.dma_start(
                    out=xlo[:],
                    in_=strip[l][g * RG:(g + 1) * RG, 0:65].rearrange(
                        "(t p) d -> p t d", p=128))
                xr64 = xr128[:, :, 0:64]
                bcol = xr128[:, :, 64:65]
            else:
                vxi = spool.tile([128, 32], i16, tag="vxi")
                gv = g - NGM
                nc.sync.dma_start(out=vxi[:], in_=P['vxidx'][:, 32 * gv:32 * (gv + 1)])
                vxr = xpool.tile([128, G, 128], f16, tag="vxr")
                nc.gpsimd.dma_gather(out_ap=vxr[:], in_ap=xr_t[l][:, :],
                                     idxs_ap=vxi[:], num_idxs=RG,
                                     num_idxs_reg=nireg(RG), elem_size=128,
                                     queue_num=nextq())
                xr64 = vxr[:, :, 0:64]
                bcol = vxr[:, :, 64:65]
            # idx loads ride the Scalar HWDGE queue so they are never stuck
            # behind Sync-queue waits; gathers fill one whole-group tile
            xlg = xlpool.tile([128, nws, 128], f16, tag="xl")
            spans = []
            qo = 0
            for c in range(NW):
                S_c = int(S[g][c])
                w = 8 * G * S_c
                it = spool.tile([128, w], i16, tag=f"it{c}")
                nc.sync.dma_start(out=it[:], in_=P[f'sidx{c}'][:, coff[c]:coff[c] + w])
                for q0 in range(0, G * S_c, 8):
                    qn = min(8, G * S_c - q0)
                    nc.gpsimd.dma_gather(
                        out_ap=xlg[:, qo + q0:qo + q0 + qn, :],
                        in_ap=table[l][c * WIN:(c + 1) * WIN, :],
                        idxs_ap=it[:, q0 * 8:(q0 + qn) * 8],
                        num_idxs=128 * qn, num_idxs_reg=nireg(128 * qn),
                        elem_size=128, queue_num=nextq())
                spans.append((qo, S_c))
                qo += G * S_c

            z = hpool.tile([128, nws, 64], f16, tag="z")
            sc = spool.tile([128, nws], f16, tag="sc")
            scn = spool.tile([128, nws], f16, tag="scn")
            ab = spool.tile([128, nws], f16, tag="ab")
            for (qo, S_c) in spans:
                z4 = z[:, qo:qo + G * S_c, :].rearrange("p (g s) d -> p g s d", g=G)
                xlg4 = xlg[:, qo:qo + G * S_c, 0:64].rearrange(
                    "p (g s) d -> p g s d", g=G)
                xrb = xr64.unsqueeze(2).to_broadcast([128, G, S_c, 64])
                nc.vector.tensor_add(z4, xlg4, xrb)
                sc3 = sc[:, qo:qo + G * S_c].rearrange("p (g s) -> p g s", g=G)
                scn3 = scn[:, qo:qo + G * S_c].rearrange("p (g s) -> p g s", g=G)
                nc.vector.tensor_reduce(sc3, z4[:, :, :, 0:NPOS],
                                        axis=mybir.AxisListType.X,
                                        op=mybir.AluOpType.add,
                                        apply_absolute_value=True)
                nc.vector.tensor_reduce(scn3, z4[:, :, :, NPOS:64],
                                        axis=mybir.AxisListType.X,
                                        op=mybir.AluOpType.add,
                                        apply_absolute_value=True)
                ab3 = ab[:, qo:qo + G * S_c].rearrange("p (g s) -> p g s", g=G)
                a3 = xlg[:, qo:qo + G * S_c, 64].rearrange("p (g s) -> p g s", g=G)
                nc.vector.tensor_add(ab3, a3,
                                     bcol[:, :, 0].unsqueeze(2).to_broadcast(
                                         [128, G, S_c]))
            nc.vector.tensor_sub(sc[:], sc[:], scn[:])
            # score = 0.4 * (sc + 1.5 * ab); exp via ACT scale
            nc.vector.scalar_tensor_tensor(
                sc[:], ab[:], 1.5, sc[:],
                mybir.AluOpType.mult, mybir.AluOpType.add)
            ex = spool.tile([128, nws], f16, tag="ex")
            nc.scalar.activation(ex[:], sc[:], mybir.ActivationFunctionType.Exp,
                                 scale=0.4)
            dtmp4 = spool.tile([128, NW, G], f16, tag="dtmp4")
            ntmp4 = spool.tile([128, NW, G, 64], f16, tag="ntmp4")
            for ci, (qo, S_c) in enumerate(spans):
                ex3 = ex[:, qo:qo + G * S_c].rearrange("p (g s) -> p g s", g=G)
                nc.vector.tensor_reduce(dtmp4[:, ci], ex3,
                                        axis=mybir.AxisListType.X,
                                        op=mybir.AluOpType.add)
                z4 = z[:, qo:qo + G * S_c, :].rearrange("p (g s) d -> p g s d", g=G)
                # expand ex across features on the Scalar engine so the DVE
                # multiply is fully packed, then sum slots by a halving tree
                # of packed adds (both ~2x faster than strided reduce)
                exw = epool.tile([128, G, S_c, 64], f16, tag="exw")
                nc.scalar.copy(exw[:], ex3.unsqueeze(3).to_broadcast(
                    [128, G, S_c, 64]))
                nc.vector.tensor_mul(z4, z4, exw[:])
                m = S_c
                while m > 2:
                    h = m // 2
                    nc.vector.tensor_add(z4[:, :, 0:h, :], z4[:, :, 0:h, :],
                                         z4[:, :, m - h:m, :])
                    m = m - h
                nt = ntmp4[:, ci].unsqueeze(2)
                if m == 2:
                    nc.vector.tensor_add(nt, z4[:, :, 0:1, :], z4[:, :, 1:2, :])
                else:
                    nc.vector.tensor_copy(nt, z4[:, :, 0:1, :])
            dtmp = spool.tile([128, G], f16, tag="dtmp")
            nc.vector.tensor_add(ntmp4[:, 0], ntmp4[:, 0], ntmp4[:, 1])
            nc.vector.tensor_add(ntmp4[:, 2], ntmp4[:, 2], ntmp4[:, 3])
            nc.vector.tensor_add(nd[:, :, 0:64], ntmp4[:, 0], ntmp4[:, 2])
            nc.vector.tensor_add(dtmp4[:, 0], dtmp4[:, 0], dtmp4[:, 1])
            nc.vector.tensor_add(dtmp4[:, 2], dtmp4[:, 2], dtmp4[:, 3])
            nc.vector.tensor_add(dtmp[:], dtmp4[:, 0], dtmp4[:, 2])
            if not is_v:
                # self-loop: z_self = xlo + xr, score from a_own + b
                zs = spool.tile([128, G, 64], f16, tag="zs")
                nc.vector.tensor_add(zs[:], xlo[:, :, 0:64], xr64)
                scs = spool.tile([128, 2, G], f16, tag="scs")
                nc.vector.tensor_reduce(scs[:, 0], zs[:, :, 0:NPOS],
                                        axis=mybir.AxisListType.X,
                                        op=mybir.AluOpType.add,
                                        apply_absolute_value=True)
                nc.vector.tensor_reduce(scs[:, 1], zs[:, :, NPOS:64],
                                        axis=mybir.AxisListType.X,
                                        op=mybir.AluOpType.add,
                                        apply_absolute_value=True)
                nc.vector.tensor_sub(scs[:, 0], scs[:, 0], scs[:, 1])
                abs_ = spool.tile([128, G], f16, tag="abs")
                nc.vector.tensor_add(abs_[:], xlo[:, :, 64], bcol[:, :, 0])
                nc.vector.scalar_tensor_tensor(
                    scs[:, 0], abs_[:], 1.5, scs[:, 0],
                    mybir.AluOpType.mult, mybir.AluOpType.add)
                exs = spool.tile([128, G], f16, tag="exs")
                nc.scalar.activation(exs[:], scs[:, 0],
                                     mybir.ActivationFunctionType.Exp, scale=0.4)
                nc.vector.tensor_add(den[:], exs[:], dtmp[:])
                # num += exs * z_self
                nc.vector.tensor_mul(zs[:], zs[:],
                                     exs[:, :].unsqueeze(2).to_broadcast([128, G, 64]))
                nc.vector.tensor_add(nd[:, :, 0:64], nd[:, :, 0:64], zs[:])
            else:
                nc.scalar.copy(den[:], dtmp[:])
            # z-trick correction: num -= den * xr
            corr = spool.tile([128, G, 64], f16, tag="corr")
            nc.vector.tensor_mul(corr[:], xr64,
                                 den[:, :].unsqueeze(2).to_broadcast([128, G, 64]))
            nc.vector.tensor_sub(nd[:, :, 0:64], nd[:, :, 0:64], corr[:])
            nc.scalar.copy(nd[:, :, 64:65], den[:, :].unsqueeze(2))
            if is_v:
                gv = g - NGM
                dst_nd = nd_v[l][gv * RG:(gv + 1) * RG, 0:65].rearrange(
                    "(t p) d -> p t d", p=128)
                nc.scalar.dma_start(out=dst_nd, in_=nd[:])
            return nd

        for l in range(NL):
            # ---- matmul phase (layer 0 only; layer 1 fused into merges) ----
            if l == 0:
                for mg in range(len(mm_groups)):
                    mm_chunk(l, mg)
            # poison row for padded slots (window-local row NS of even strip)
            nc.sync.dma_start(out=strip[l][NS:NS + 1, :], in_=poison[:])
            # ---- all-gather the table ----
            nc.gpsimd.collective_compute(
                "AllGather", mybir.AluOpType.bypass,
                replica_groups=[list(range(M))],
                ins=[strip[l][0:PC, :]], outs=[table[l][:, :]])

            # ---- slot offsets ----
            coff = [0] * NW
            coff_at = {}
            for g in range(NG):
                coff_at[g] = list(coff)
                for c in range(NW):
                    coff[c] += 8 * G * int(S[g][c])
            # ---- virtual groups first, then main + fused merge (+ next mm) --
            for g in range(NGM, NG):
                slot_group(l, g, coff_at[g])
            for g in range(NGM):
                nd = slot_group(l, g, coff_at[g])
                xt = merge_group(l, g, nd, HV[g])
                if l == 0 and NL > 1:
                    mm_chunk(1, g, xt_in=xt)

    nc.compile()
    return nc


# ----------------------------------------------------------------------
# entry point
# ----------------------------------------------------------------------

def kernel(**inputs):
    x = np.asarray(inputs['x'], np.float32)
    ei = np.asarray(inputs['edge_index'])
    W_l = np.asarray(inputs['W_l'], np.float64)
    b_l = np.asarray(inputs['b_l'], np.float64)
    W_r = np.asarray(inputs['W_r'], np.float64)
    b_r = np.asarray(inputs['b_r'], np.float64)
    att = np.asarray(inputs['att'], np.float64)
    bias = np.asarray(inputs['bias'], np.float64)

    T = preprocess(x, ei)

    # fold |att| into the weights; permute features so att>0 comes first
    Pm = np.concatenate([np.where(att > 0)[0], np.where(att <= 0)[0]])
    NPOS = int((att > 0).sum())
    aab = np.abs(att[Pm])
    aab[aab == 0] = 1.0

    nc = build_program(T['S'], T['NGV'], NPOS, T['hv'])

    def wcat_of(Wl, bl, Wr, br):
        Wc = np.zeros((65, 128), np.float32)
        Wc[:64, 0:64] = Wl
        Wc[64, 0:64] = bl
        Wc[:64, 64:128] = Wr
        Wc[64, 64:128] = br
        return Wc

    Wl1 = W_l[:, Pm] * aab; bl1 = b_l[Pm] * aab
    Wr1 = W_r[:, Pm] * aab; br1 = b_r[Pm] * aab
    Wl2 = (W_l[Pm][:, Pm] * aab) / aab[:, None]
    Wr2 = (W_r[Pm][:, Pm] * aab) / aab[:, None]
    Wcat0 = wcat_of(Wl1, bl1, Wr1, br1)
    Wcat1 = wcat_of(Wl2, bl1, Wr2, br1)
    bias0 = np.tile((bias[Pm] * aab)[None, :], (128, 1)).astype(np.float32)
    bias1 = np.tile(bias[Pm][None, :], (128, 1)).astype(np.float32)
    rat = np.tile((1.0 / aab)[None, :], (128, 1)).astype(np.float32)
    ident = np.eye(128, dtype=np.float32)
    poison = np.zeros((1, 128), np.float16)
    poison[0, 64] = POISON_A

    in_maps = []
    for m in range(M):
        xT0 = np.zeros((65, PC), np.float16)
        xT0[0:64, 0:NS] = x[T['node_order'][m]].T
        xT0[64, :] = 1.0
        pcm = T['percore'][m]
        im = dict(xT0=xT0, Wcat0=Wcat0.astype(np.float16), Wcat1=Wcat1, bias0=bias0, bias1=bias1,
                  rat=rat, ident=ident, poison=poison,
                  vxidx=pcm['vxidx'], mB=pcm['mB'])
        for c in range(NW):
            im[f'sidx{c}'] = pcm['sidx'][c]
        in_maps.append(im)

    if os.environ.get('BASS_GAT_SIM'):
        from concourse import bass_interp
        sim = bass_interp.MultiCoreSim(nc, M)
        for m in range(M):
            for k, v in in_maps[m].items():
                sim.cores[m].tensor(k)[:] = v
        sim.simulate()
        results = [{'h2': sim.cores[m].mem_tensor('h2')} for m in range(M)]
    else:
        from concourse.bass_utils import run_bass_kernel_spmd
        trace = bool(os.environ.get('BASS_GAT_TRACE'))
        res = run_bass_kernel_spmd(nc, in_maps, list(range(M)), trace=trace)
        if trace:
            print(f"[hw] exec_time_ns: {res.exec_time_ns}")
            print(f"HW exec time: {res.exec_time_ns} ns")
        results = res.results

    h2 = np.zeros((N, D), np.float32)
    for m in range(M):
        h2[np.ix_(T['node_order'][m], Pm)] = results[m]['h2'][:NS]
    return x + h2
